# revision 5
# baseline (speedup 1.0000x reference)
"""Trainium2 Bass kernel for nn_Corssattention (dense_transformer), v2.

Full inputs in, full outputs out. Sharding: data-parallel over batch B=8,
one sample per NeuronCore, weights replicated, no cross-core comm.

v2 over baseline (260.6us -> 219.8us modeled):
- The big bf16 matmul groups move to fp8 DoubleRow (0.5 cyc/row) with
  host-side hi/lo weight splits (whi=f8(w), wlo=f8(w-whi); one DROW pass
  per term keeps ~10-bit effective weight precision):
  * dw 3x3: 9 DROW passes, tap hi/lo stationary pairs; the moving AP is
    the fp8 padded image window with a stride-0 subtile broadcast dim
    (hand-built 4D AP) instead of 9 bf16 diag passes
  * fc1: u in fp8, w1 hi/lo -> 2 DROW passes (was 2 bf16 passes)
  * fc2: w2 hi/lo -> 8 DROW passes (bf16-grade weights, was single fp8)
  * q-proj from z8 via DROW
- mlp-Y pulled ahead of the x loop: u_y = y1*ax only needs stream-0
  attention, so it fills the chain(1) serial latency; y3 goes to DRAM and
  is re-read on the same sync queue (FIFO-ordered) for the x3 adds.
- z8/Q/KT/KT8/u8 split into column-chunk tiles to break tile-granular
  false dependencies at phase boundaries; y input load interleaved into
  sepconv_x as its zbf chunks free up.
- Drains rebalanced: relu+bias on Act, min6 on DVE (2 els/cyc fp8 SBUF),
  u-mults/x3t-add on Pool (no PSUM access allowed there), residual DMAs
  on the SP HWDGE queue instead of Pool SWDGE descriptors.
"""

import numpy as np
import ml_dtypes

import concourse.bass as bass
from concourse import bacc
import concourse.mybir as mybir
import concourse.tile as tile
from concourse.bass_utils import run_bass_kernel_spmd
import bass_rust

F32 = mybir.dt.float32
BF16 = mybir.dt.bfloat16
FP8 = mybir.dt.float8e4
AX = mybir.AxisListType.X
OP = mybir.AluOpType
ACTF = mybir.ActivationFunctionType
DROW = mybir.MatmulPerfMode.DoubleRow

C = 256
H = W = 64
N = H * W            # 4096
M = 32               # KAttention proj dim
HID = 1024
NT512 = N // 512     # 8
NT128 = N // 128     # 32
EPS = 1e-6
PADW = 66            # 64 + 1 pad each side

BF = ml_dtypes.bfloat16
F8 = ml_dtypes.float8_e4m3

TAPS = [(dy, dx) for dy in (-1, 0, 1) for dx in (-1, 0, 1)]

_CACHE = {}


def _build(gk: float, gc: float):
    nc = bacc.Bacc("TRN2", target_bir_lowering=False, debug=False)

    d = {}
    def din(name, shape, dt):
        d[name] = nc.dram_tensor(name, list(shape), dt, kind="ExternalInput").ap()
    din("xb", (C, N), F32)
    din("yb", (C, N), F32)
    din("pwT", (C, C), BF16)          # conv pw weight, transposed [cin, cout]
    din("dwd8", (128, 36, 128), FP8)  # diag dw [p, ((mh*9+tap)*2+hl), m]
    din("zbc", (128, 2), F32)         # z1 bias as per-partition cols
    din("tv8", (128, 2, 512), FP8)    # [I256 | v_w.T], kt-subtile layout
    din("kT", (C, M), BF16)
    din("qT8", (128, 2, M), FP8)
    din("w1h8", (128, 2, HID), FP8)   # fc1 weights hi, kt-subtile layout
    din("w1l8", (128, 2, HID), FP8)   # fc1 weights lo
    din("b1c", (128, 8), F32)         # fc1 bias cols per hm block
    din("w2h8", (128, 8, C), FP8)     # fc2 weights hi, k-subtile layout
    din("w2l8", (128, 8, C), FP8)     # fc2 weights lo
    din("b2c", (128, 2), F32)         # fc2 bias cols per mh half
    din("i128f", (128, 128), F32)
    din("i128b", (128, 128), BF16)
    din("o128f8", (128, 1), FP8)
    din("ones32", (1, M), BF16)
    x3o = nc.dram_tensor("x3o", [C, N], F32, kind="ExternalOutput").ap()
    y3o = nc.dram_tensor("y3o", [C, N], F32, kind="ExternalOutput").ap()

    with tile.TileContext(nc) as tc:
        with tc.tile_pool(name="wp", bufs=1) as wp, \
             tc.tile_pool(name="tpz", bufs=2) as tpz, \
             tc.tile_pool(name="tvc", bufs=4) as tvcp, \
             tc.tile_pool(name="tio", bufs=3) as tio, \
             tc.tile_pool(name="tms", bufs=2) as tms, \
             tc.tile_pool(name="hp", bufs=8) as hp, \
             tc.tile_pool(name="psA", bufs=4, space="PSUM") as psA, \
             tc.tile_pool(name="psL", bufs=2, space="PSUM") as psL, \
             tc.tile_pool(name="psR", bufs=2, space="PSUM") as psR:

            # ---- stream-x input load first (startup long pole), chunked.
            zbf = [[wp.tile([128, N // 4], BF16, tag=f"zbf{h}{q}",
                            name=f"zbf{h}{q}") for q in range(4)]
                   for h in range(2)]

            def load_chunk(z_dram, q, dma_eng, cv_eng):
                for h in range(2):
                    zf = tpz.tile([128, N // 4], F32, tag="zf")
                    sl = slice(q * (N // 4), (q + 1) * (N // 4))
                    dma_eng.dma_start(out=zf[:],
                                      in_=z_dram[h*128:(h+1)*128, sl])
                    if cv_eng is nc.scalar:
                        nc.scalar.copy(out=zbf[h][q][:], in_=zf[:])
                    else:
                        cv_eng.tensor_copy(zbf[h][q][:], zf[:])

            def load_input(z_dram, eng=None):
                qengs = ([nc.scalar, nc.gpsimd, nc.sync] if eng is None
                         else [nc.sync])
                for q in range(4):
                    load_chunk(z_dram, q, qengs[q % len(qengs)],
                               eng or nc.vector)

            def wtile(name, shape, dt, src):
                t = wp.tile(list(shape), dt, tag=name, name=name)
                nc.sync.dma_start(out=t[:], in_=src)
                return t

            pw_sb = [wtile(f"pw{k}", (128, C), BF16, d["pwT"][k*128:(k+1)*128, :])
                     for k in range(2)]

            load_input(d["xb"])

            # ---- remaining weights ----
            dwd_sb = wtile("dwd8", (128, 36, 128), FP8, d["dwd8"][:, :, :])
            zbc_sb = wtile("zbc", (128, 2), F32, d["zbc"][:, :])
            tv8_sb = wtile("tv8", (128, 2, 512), FP8, d["tv8"][:, :, :])
            kT_sb = [wtile(f"kT{k}", (128, M), BF16, d["kT"][k*128:(k+1)*128, :])
                     for k in range(2)]
            qT8_sb = wtile("qT8", (128, 2, M), FP8, d["qT8"][:, :, :])
            w1h_sb = wtile("w1h8", (128, 2, HID), FP8, d["w1h8"][:, :, :])
            w1l_sb = wtile("w1l8", (128, 2, HID), FP8, d["w1l8"][:, :, :])
            b1c_sb = wtile("b1c", (128, 8), F32, d["b1c"][:, :])
            w2h_sb = wtile("w2h8", (128, 8, C), FP8, d["w2h8"][:, :, :])
            w2l_sb = wtile("w2l8", (128, 8, C), FP8, d["w2l8"][:, :, :])
            b2c_sb = wtile("b2c", (128, 2), F32, d["b2c"][:, :])
            idf_sb = wtile("i128f", (128, 128), F32, d["i128f"][:, :])
            idb_sb = wtile("i128b", (128, 128), BF16, d["i128b"][:, :])
            o128_sb = wtile("o128f8", (128, 1), FP8, d["o128f8"][:, :])
            o32_sb = wtile("ones32", (1, M), BF16, d["ones32"][:, :])

            # ---- persistent tensors ----
            ppad = [wp.tile([128, PADW, PADW], FP8, tag=f"ppad{h}",
                            name=f"ppad{h}") for h in range(2)]
            z1 = {s: [wp.tile([128, N], BF16, tag=f"z1{s}{h}", name=f"z1{s}{h}")
                      for h in range(2)] for s in range(2)}
            z8 = [[wp.tile([128, 2, N // 4], FP8, tag=f"z8{s}c{c}",
                           name=f"z8{s}c{c}") for c in range(4)]
                  for s in range(2)]

            def z8ap(s, lo, size):
                c, off = divmod(lo, N // 4)
                assert off + size <= N // 4
                return z8[s][c][:, :, off:off+size]
            u8 = [[wp.tile([128, 2, 512], FP8, tag=f"u8{s}n{nt}",
                           name=f"u8{s}n{nt}") for nt in range(NT512)]
                  for s in range(2)]
            ax = [wp.tile([128, N], BF16, tag=f"ax{h}", name=f"ax{h}")
                  for h in range(2)]
            KT = [[wp.tile([128, 8, M], BF16, tag=f"KT{s}g{g}",
                           name=f"KT{s}g{g}") for g in range(4)]
                  for s in range(2)]                       # exp intermediate
            KT8 = [[wp.tile([128, 8, M], FP8, tag=f"KT8{s}g{g}",
                            name=f"KT8{s}g{g}") for g in range(4)]
                   for s in range(2)]                      # softplus, fp8
            Q = [[wp.tile([M, N // 4], BF16, tag=f"Q{s}g{g}",
                          name=f"Q{s}g{g}") for g in range(4)]
                 for s in range(2)]

            def Qap(s, lo, size):
                g, off = divmod(lo, N // 4)
                assert off + size <= N // 4
                return Q[s][g][:, off:off+size]
            invFM_t = wp.tile([1, N], BF16, tag="invFM", name="invFM")
            invFM = [invFM_t, invFM_t]

            # zero pad cells of ppad once (interior overwritten each stream)
            for h in range(2):
                p = ppad[h]
                nc.gpsimd.memset(p[:, 0, :], 0.0)
                nc.gpsimd.memset(p[:, PADW - 1, :], 0.0)
                nc.gpsimd.memset(p[:, :, 0], 0.0)
                nc.gpsimd.memset(p[:, :, PADW - 1], 0.0)

            def dw_rhs(mh, nt, dy, dx):
                """4D moving AP over ppad fp8: [128, 2(bcast), 8, 64] window
                at tap (dy,dx) for output row block nt (8 rows)."""
                a = ppad[mh][:]
                off = a.offset + (1 + nt * 8 + dy) * PADW + (1 + dx)
                return bass_rust.AP(a.tensor, off,
                                    [[PADW * PADW, 128], [0, 2], [PADW, 8],
                                     [1, 64]])

            def pw_unit(mh, nt, alt_copy=False):
                ps = psA.tile([128, 512], F32, tag="mm")
                q, r = divmod(nt, 2)
                for kt in range(2):
                    nc.tensor.matmul(
                        ps[:], pw_sb[kt][:, mh*128:(mh+1)*128],
                        zbf[kt][q][:, r*512:(r+1)*512],
                        start=(kt == 0), stop=(kt == 1))
                h0 = nt * 8
                dst = ppad[mh][:, 1+h0:1+h0+8, 1:65]
                src = ps[:].rearrange("p (h w) -> p h w", h=8)
                if alt_copy and mh == 1:
                    nc.vector.tensor_copy(dst, src)
                else:
                    nc.scalar.copy(out=dst, in_=src)

            def dw_unit(s, mh, nt):
                """depthwise 3x3: 9 fp8 DROW passes, weight hi/lo subtile
                pairs, moving = broadcast-subtile window views of ppad."""
                ps = psA.tile([128, 512], F32, tag="mm")
                for t, (dy, dx) in enumerate(TAPS):
                    st = dwd_sb[:, (mh*9 + t)*2:(mh*9 + t)*2 + 2, :]
                    nc.tensor.matmul(
                        ps[:], st, dw_rhs(mh, nt, dy, dx),
                        start=(t == 0), stop=(t == 8), perf_mode=DROW)
                sl = slice(nt*512, (nt+1)*512)
                if s == 0 or nt % 2 == 0:
                    nc.vector.tensor_scalar(
                        z1[s][mh][:, sl], ps[:], zbc_sb[:, mh:mh+1], None,
                        OP.add)
                else:
                    nc.scalar.activation(
                        out=z1[s][mh][:, sl], in_=ps[:], func=ACTF.Identity,
                        bias=zbc_sb[:, mh:mh+1])
                # Pool cannot read PSUM: derive z8 from z1 (SBUF->SBUF)
                c, off = divmod(nt*512, N // 4)
                nc.gpsimd.tensor_copy(z8[s][c][:, mh, off:off+512],
                                      z1[s][mh][:, sl])

            def q_proj(s, qt):
                pq = psA.tile([M, 512], F32, tag="mm")
                nc.tensor.matmul(pq[:], qT8_sb[:], z8ap(s, qt*512, 512),
                                 start=True, stop=True, perf_mode=DROW)
                nc.scalar.activation(out=Qap(s, qt*512, 512),
                                     in_=pq[:], func=ACTF.Exp)

            def k_group(s, g):
                pk = psR.tile([128, 4, M], F32, tag="rt")
                for j in range(4):
                    nt = 4*g + j
                    for kt in range(2):
                        nc.tensor.matmul(pk[:, j, :],
                                         z1[s][kt][:, nt*128:(nt+1)*128],
                                         kT_sb[kt][:],
                                         start=(kt == 0), stop=(kt == 1))
                nc.scalar.activation(
                    out=KT[s][4*g // 8][:, (4*g) % 8:(4*g) % 8 + 4, :],
                    in_=pk[:], func=ACTF.Exp)

            def sepconv_x():
                cv = [nc.vector, nc.vector, nc.vector, nc.vector]
                for nt in range(NT512):
                    for mh in range(2):
                        pw_unit(mh, nt)
                    if nt % 2 == 1:
                        q = nt // 2
                        load_chunk(d["yb"], q, nc.sync, cv[q])
                    if nt >= 1:
                        for mh in range(2):
                            dw_unit(0, mh, nt - 1)
                    if nt >= 2:
                        q_proj(0, nt - 2)
                        k_group(0, nt - 2)
                for mh in range(2):
                    dw_unit(0, mh, NT512 - 1)
                for qt in range(NT512 - 2, NT512):
                    q_proj(0, qt)
                    k_group(0, qt)

            def sepconv_dw1():
                for nt in range(NT512):
                    for mh in range(2):
                        dw_unit(1, mh, nt)
                    if nt >= 1:
                        q_proj(1, nt - 1)
                        k_group(1, nt - 1)
                q_proj(1, NT512 - 1)
                k_group(1, NT512 - 1)

            def light_a(s, st, filler=None, prefill=None):
                if prefill is not None:
                    prefill()
                with tc.tile_critical():
                    for g in range(4):
                        nc.scalar.activation(
                            out=KT8[s][g][:], in_=KT[s][g][:],
                            func=ACTF.Ln, bias=1.0)
                    for g in range(4):
                        nc.scalar.activation(
                            out=Q[s][g][:], in_=Q[s][g][:],
                            func=ACTF.Ln, bias=1.0)
                pe = [psL.tile([128, C], F32, tag="lv", name=f"pe{i}")
                      for i in range(2)]
                pkv = psR.tile([M, C], F32, tag="rt")
                for pr in range(16):
                    if filler is not None:
                        filler(pr)
                    chp = tvcp.tile([128, 2, 512], FP8, tag="tvc")
                    for j in range(2):
                        nt = 2*pr + j
                        ps = psA.tile([128, 512], F32, tag="mm")
                        nc.tensor.matmul(ps[:], z8ap(s, nt*128, 128),
                                         tv8_sb[:], start=True, stop=True,
                                         perf_mode=DROW)
                        if j == 0:
                            nc.vector.tensor_copy(chp[:, j, :], ps[:])
                        else:
                            nc.scalar.copy(out=chp[:, j, :], in_=ps[:])
                    for mh in range(2):
                        nc.tensor.matmul(pe[mh][:],
                                         chp[:, :, mh*128:(mh+1)*128],
                                         chp[:, :, 0:256],
                                         start=(pr == 0), stop=(pr == 15),
                                         perf_mode=DROW)
                    nc.tensor.matmul(
                        pkv[:],
                        KT8[s][(2*pr) // 8][:, (2*pr) % 8:(2*pr) % 8 + 2, :],
                        chp[:, :, 256:512],
                        start=(pr == 0), stop=(pr == 15),
                        perf_mode=DROW)
                st["pe"], st["pkv"] = pe, pkv

            def light_chain(s, st):
                pe, pkv = st["pe"], st["pkv"]
                kv = tms.tile([M, C], BF16, tag="kv", bufs=2)
                nc.vector.tensor_scalar(kv[:], pkv[:], gk, None, OP.mult)
                st["kv"] = kv
                pks = psR.tile([M, 1], F32, tag="rt")
                for nt in range(NT128):
                    nc.tensor.matmul(pks[:],
                                     KT8[s][nt // 8][:, nt % 8, :],
                                     o128_sb[:],
                                     start=(nt == 0), stop=(nt == NT128 - 1))
                ksum = tms.tile([M, 1], BF16, tag="ksum")
                nc.vector.tensor_scalar(ksum[:], pks[:], EPS, None, OP.add)
                pdT = psR.tile([128, NT128], F32, tag="rt")
                for t in range(NT128):
                    nc.tensor.matmul(pdT[:, t:t+1], Qap(s, t*128, 128),
                                     ksum[:], start=True, stop=True)
                invA = tms.tile([128, NT128], F32, tag="invA")
                nc.vector.reciprocal(invA[:], pdT[:])
                piT = psR.tile([M, 128], F32, tag="rt")
                nc.tensor.transpose(piT[:], invA[:], idf_sb[:])
                invT = tms.tile([NT128, 128], BF16, tag="invT")
                nc.vector.tensor_copy(invT[:], piT[:])
                nc.gpsimd.dma_start(out=invFM[s][0:1, :], in_=invT[:])
                att = [tms.tile([128, C], BF16, tag=f"att{mh}", name=f"att{mh}")
                       for mh in range(2)]
                for mh in range(2):
                    rmax = tms.tile([128, 1], F32, tag="rmax")
                    nc.vector.tensor_reduce(rmax[:], pe[mh][:], axis=AX,
                                            op=OP.min)
                    tdiff = tms.tile([128, C], F32, tag="tdiff")
                    nc.vector.tensor_scalar(tdiff[:], pe[mh][:], rmax[:], 30.0,
                                            OP.subtract, OP.min)
                    ex = tms.tile([128, C], F32, tag="ex")
                    nc.scalar.activation(out=ex[:], in_=tdiff[:], func=ACTF.Exp,
                                         scale=-1.0)
                    rsum = tms.tile([128, 1], F32, tag="rsum")
                    nc.vector.reduce_sum(rsum[:], ex[:], axis=AX)
                    rinv = tms.tile([128, 1], F32, tag="rinv")
                    nc.vector.reciprocal(rinv[:], rsum[:])
                    nc.vector.tensor_scalar(att[mh][:], ex[:], rinv[:], gc,
                                            OP.mult, OP.mult)
                att8T = tms.tile([128, 2, C], FP8, tag="att8T", bufs=2)
                for mh in range(2):
                    for kt in range(2):
                        pat = psR.tile([128, 128], BF16, tag="rt")
                        nc.tensor.transpose(pat[:],
                                            att[mh][:, kt*128:(kt+1)*128],
                                            idb_sb[:])
                        nc.vector.tensor_copy(
                            att8T[:, kt, mh*128:(mh+1)*128], pat[:])
                st["att8T"] = att8T

            def light_post_piece(s, nt):
                pib = psA.tile([M, 512], F32, tag="mm")
                nc.tensor.matmul(pib[:], o32_sb[0:1, :],
                                 invFM[s][0:1, nt*512:(nt+1)*512],
                                 start=True, stop=True)
                nc.vector.tensor_tensor(Qap(s, nt*512, 512),
                                        Qap(s, nt*512, 512), pib[:],
                                        OP.mult)

            def light_chain_post(s, st):
                for nt in range(NT512):
                    light_post_piece(s, nt)

            def light_b(s, st, nts):
                """wv + catt in one PSUM; combine a = 2*z1 + psum.
                s=0: store into ax.  s=1: ay consumed on the fly ->
                u8[0] = fp8(x1*ay), u8[1] = fp8(y1*ax)."""
                z = z1[s]
                kv, att8T = st["kv"], st["att8T"]
                for nt in nts:
                    pos = []
                    for mh in range(2):
                        po = psA.tile([128, 512], F32, tag="mm")
                        nc.tensor.matmul(po[:], kv[:, mh*128:(mh+1)*128],
                                         Qap(s, nt*512, 512),
                                         start=True, stop=False)
                        nc.tensor.matmul(po[:],
                                         att8T[:, :, mh*128:(mh+1)*128],
                                         z8ap(s, nt*512, 512),
                                         start=False, stop=True,
                                         perf_mode=DROW)
                        pos.append(po)
                    for mh in range(2):
                        sl = slice(nt*512, (nt+1)*512)
                        if s == 0:
                            nc.vector.scalar_tensor_tensor(
                                ax[mh][:, sl], z[mh][:, sl], 2.0, pos[mh][:],
                                OP.mult, OP.add)
                            # u_y = y1*ax ready as soon as ax lands (Pool,
                            # idle in this phase) -> mlp-Y can run early
                            nc.gpsimd.tensor_tensor(u8[1][nt][:, mh, :],
                                                    z1[1][mh][:, sl],
                                                    ax[mh][:, sl], OP.mult)
                        else:
                            ay_t = tvcp.tile([128, 512], BF16, tag="ayt")
                            nc.vector.scalar_tensor_tensor(
                                ay_t[:], z[mh][:, sl], 2.0, pos[mh][:],
                                OP.mult, OP.add)
                            nc.gpsimd.tensor_tensor(u8[0][nt][:, mh, :],
                                                    z1[0][mh][:, sl], ay_t[:],
                                                    OP.mult)

            # mlp pieces -----------------------------------------------------
            def mlp_fc1(s, nt, min6_dve=3):
                """hidden = relu6(fc1(u8) + b): 2 DROW passes per 128-block
                (w1 hi + lo); relu+bias drains split Act/DVE, min6 split
                DVE/Pool (first `min6_dve` p's on DVE)."""
                hps = []
                for p in range(4):
                    h8 = hp.tile([128, 2, 512], FP8, tag="h")
                    for i in range(2):
                        hm = 2*p + i
                        ph = psA.tile([128, 512], F32, tag="mm")
                        for wsb in (w1h_sb, w1l_sb):
                            nc.tensor.matmul(
                                ph[:], wsb[:, :, hm*128:(hm+1)*128],
                                u8[s][nt][:],
                                start=(wsb is w1h_sb), stop=(wsb is w1l_sb),
                                perf_mode=DROW)
                        nc.scalar.activation(out=h8[:, i, :], in_=ph[:],
                                             func=ACTF.Relu,
                                             bias=b1c_sb[:, hm:hm+1])
                    if p < min6_dve:
                        nc.vector.tensor_scalar(h8[:], h8[:], 6.0, None,
                                                OP.min)
                    else:
                        nc.gpsimd.tensor_scalar(h8[:], h8[:], 6.0, None,
                                                OP.min)
                    hps.append(h8)
                return hps

            def mlp_fc2(hps):
                outs = []
                for mh in range(2):
                    p2 = psL.tile([128, 512], F32, tag="lv")
                    for p in range(4):
                        for k, wsb in enumerate((w2h_sb, w2l_sb)):
                            nc.tensor.matmul(
                                p2[:], wsb[:, 2*p:2*p+2, mh*128:(mh+1)*128],
                                hps[p][:], start=(p == 0 and k == 0),
                                stop=(p == 3 and k == 1),
                                perf_mode=DROW)
                    outs.append(p2)
                return outs

            def mlp_y_unit(nt):
                """y2 = mlp(u_y); y3 = y + y2 -> DRAM (re-read in the x loop,
                same sync queue so the round trip is ordered)."""
                hs_y = mlp_fc1(1, nt, min6_dve=4)
                y2p = mlp_fc2(hs_y)
                for mh in range(2):
                    yin = tio.tile([128, 512], F32, tag="zin", bufs=4)
                    nc.sync.dma_start(out=yin[:],
                                      in_=d["yb"][mh*128:(mh+1)*128,
                                                  nt*512:(nt+1)*512])
                    y3t = tio.tile([128, 512], F32, tag="y3t", bufs=4)
                    nc.vector.scalar_tensor_tensor(
                        y3t[:], y2p[mh][:], b2c_sb[:, mh:mh+1], yin[:],
                        OP.add, OP.add)
                    nc.sync.dma_start(out=y3o[mh*128:(mh+1)*128,
                                              nt*512:(nt+1)*512], in_=y3t[:])

            st0, st1 = {}, {}
            sepconv_x()
            light_a(0, st0,
                    filler=lambda u: (pw_unit((u // 2) // 4,
                                              4 + (u // 2) % 4, alt_copy=True)
                                      if u % 2 == 0 else None),
                    prefill=lambda: [pw_unit(mh, nt, alt_copy=True)
                                     for mh in range(2) for nt in range(4)])
            light_chain(0, st0)
            sepconv_dw1()              # PE filler for chain(0) latency
            light_chain_post(0, st0)
            light_a(1, st1,
                    filler=lambda u: (light_b(0, st0, [u // 4 + 3])
                                      if u % 4 == 0 and u < 12 else None),
                    prefill=lambda: light_b(0, st0, range(0, 3)))
            light_b(0, st0, range(6, NT512))
            light_chain(1, st1)
            for nt in range(NT512):    # mlp-Y fills chain(1) latency
                mlp_y_unit(nt)
                light_post_piece(1, nt)

            # x-stream mlp + final adds, tile by tile
            for step in range(NT512 + 1):
                if step < NT512:
                    light_b(1, st1, [step])
                if step == 0:
                    continue
                nt = step - 1
                ins = []
                for mh in range(2):
                    y3in = tio.tile([128, 512], F32, tag="zin", bufs=4)
                    nc.sync.dma_start(out=y3in[:],
                                      in_=y3o[mh*128:(mh+1)*128,
                                              nt*512:(nt+1)*512])
                    xin = tio.tile([128, 512], F32, tag="zin", bufs=4)
                    nc.sync.dma_start(out=xin[:],
                                      in_=d["xb"][mh*128:(mh+1)*128,
                                                  nt*512:(nt+1)*512])
                    ins.append((y3in, xin))
                hs_x = mlp_fc1(0, nt, min6_dve=4)
                x2p = mlp_fc2(hs_x)
                for mh in range(2):
                    y3in, xin = ins[mh]
                    x3t = tio.tile([128, 512], F32, tag="x3t")
                    nc.vector.scalar_tensor_tensor(
                        x3t[:], x2p[mh][:], b2c_sb[:, mh:mh+1], y3in[:],
                        OP.add, OP.add)
                    nc.gpsimd.tensor_tensor(x3t[:], x3t[:], xin[:], OP.add)
                    nc.sync.dma_start(out=x3o[mh*128:(mh+1)*128,
                                              nt*512:(nt+1)*512], in_=x3t[:])
    nc.compile()
    return nc


def _hilo(a):
    hi = np.clip(a, -240.0, 240.0).astype(F8)
    lo = (a - hi.astype(np.float32)).astype(F8)
    return hi, lo


def _prep_weights(i):
    """Host-side weight folding; returns dict of DMA-ready arrays."""
    bf = lambda a: np.ascontiguousarray(a).astype(BF)
    f8 = lambda a: np.ascontiguousarray(a).astype(F8)
    f32 = lambda a: np.ascontiguousarray(a, dtype=np.float32)

    pw = f32(i["conv_pw_w"])            # [C, C]  out,in
    dw = f32(i["conv_dw_w"])[:, 0]      # [C, 3, 3]
    cs, ct = f32(i["conv_bn_s"]), f32(i["conv_bn_t"])
    ns, nt_ = f32(i["norm_s"]), f32(i["norm_t"])
    dwf = dw * (cs * ns)[:, None, None]                    # fold BN+norm scale
    zbias = ct * ns + nt_                                  # fold BN+norm shift
    dwd = np.zeros((128, 36, 128), np.float32)
    for half in range(2):
        for tap, (dy, dx) in enumerate(TAPS):
            w = dwf[half*128:(half+1)*128, dy+1, dx+1]
            whi, wlo = _hilo(w)
            base = (half*9 + tap) * 2
            dwd[np.arange(128), base, np.arange(128)] = whi.astype(np.float32)
            dwd[np.arange(128), base+1, np.arange(128)] = wlo.astype(np.float32)
    tv = np.concatenate([np.eye(C, dtype=np.float32), f32(i["v_w"]).T], axis=1)
    tv8 = tv.reshape(2, 128, 512).transpose(1, 0, 2)       # [128, kt, 512]
    s1, t1 = f32(i["bn1_s"]), f32(i["bn1_t"])
    s2, t2 = f32(i["bn2_s"]), f32(i["bn2_t"])
    fc1, fb1 = f32(i["fc1_w"]), f32(i["fc1_b"])
    fc2, fb2 = f32(i["fc2_w"]), f32(i["fc2_b"])
    w1f = (s1[:, None] * fc1) * ns[None, :]
    b1f = s1 * (fc1 @ nt_ + fb1) + t1
    w2f = s2[:, None] * fc2
    b2f = s2 * fb2 + t2

    w1T = w1f.T.reshape(2, 128, HID).transpose(1, 0, 2)    # [128, kt, HID]
    w1h, w1l = _hilo(w1T)
    w2T = w2f.T.reshape(8, 128, C).transpose(1, 0, 2)      # [128, k8, C]
    w2h, w2l = _hilo(w2T)
    qT8 = f32(i["q_w"]).T.reshape(2, 128, M).transpose(1, 0, 2)

    return {
        "pwT": bf(pw.T),
        "dwd8": f8(dwd),
        "zbc": f32(zbias.reshape(2, 128).T),
        "tv8": f8(tv8),
        "kT": bf(f32(i["k_w"]).T),
        "qT8": f8(qT8),
        "w1h8": w1h, "w1l8": w1l,
        "b1c": f32(b1f.reshape(8, 128).T),
        "w2h8": w2h, "w2l8": w2l,
        "b2c": f32(b2f.reshape(2, 128).T),
        "i128f": np.eye(128, dtype=np.float32),
        "i128b": bf(np.eye(128)),
        "o128f8": f8(np.ones((128, 1))),
        "ones32": bf(np.ones((1, M))),
    }


def kernel(**inputs):
    x = np.ascontiguousarray(inputs["x"], dtype=np.float32)
    y = np.ascontiguousarray(inputs["y"], dtype=np.float32)
    B = x.shape[0]
    gk = float(np.asarray(inputs["gamma_k"]).reshape(-1)[0])
    gc = float(np.asarray(inputs["gamma_c"]).reshape(-1)[0])

    wmaps = _prep_weights(inputs)
    key = (gk, gc) + tuple(hash(v.tobytes()) for _, v in sorted(wmaps.items()))
    if key not in _CACHE:
        _CACHE.clear()
        _CACHE[key] = _build(gk, gc)
    nc = _CACHE[key]

    in_maps = []
    for b in range(B):
        m = dict(wmaps)
        m["xb"] = x[b].reshape(C, N)
        m["yb"] = y[b].reshape(C, N)
        in_maps.append(m)

    res = run_bass_kernel_spmd(nc, in_maps, list(range(B)))
    x3 = np.stack([res.results[b]["x3o"].reshape(C, H, W) for b in range(B)])
    y3 = np.stack([res.results[b]["y3o"].reshape(C, H, W) for b in range(B)])
    return (x3.astype(np.float32), y3.astype(np.float32))


# revision 7
# speedup vs baseline: 1.0426x; 1.0426x over previous
"""Trainium2 Bass kernel for nn_Corssattention (dense_transformer), v2.

Full inputs in, full outputs out. Sharding: data-parallel over batch B=8,
one sample per NeuronCore, weights replicated, no cross-core comm.

v2 over baseline (260.6us -> 210.8us modeled):
- Big bf16 matmul groups moved to fp8 DoubleRow (0.5 cyc/row) with
  host-side hi/lo weight splits (whi=f8(w), wlo=f8(w-whi); one DROW pass
  per term keeps ~10-bit weight precision): dw 3x3 as 9 DROW passes with
  a stride-0 broadcast-subtile 4D moving AP over the fp8 padded image;
  fc1 with fp8 u and hi/lo w1 (2 passes); fc2 with hi/lo w2 (8 passes,
  more accurate than the old single-fp8 w2); q-proj from z8.
- Inputs also uploaded as host-cast bf16 (xb16/yb16): no on-chip f32->
  bf16 conversion passes, half the startup DMA bytes; f32 copies still
  used for the final residual adds.
- mlp-Y hoisted ahead of the x loop (u_y = y1*ax needs only stream-0
  attention) to fill the stream-1 chain latency; y3 round-trips through
  DRAM on the FIFO-ordered sync queue for the x3 adds.
- z8/Q/KT/KT8/u8 split into column-chunk tiles to break tile-granular
  false deps; y-input load interleaved into sepconv_x as zbf frees up.
- Drains balanced per phase: relu+bias on Act, min6 on DVE (+Pool in the
  Y phase), u-mults/x3t-add on Pool (which cannot touch PSUM), residual
  DMAs on the SP HWDGE queue; deeper h8/zin/tvc pools for pipelining.
"""

import numpy as np
import ml_dtypes

import concourse.bass as bass
from concourse import bacc
import concourse.mybir as mybir
import concourse.tile as tile
from concourse.bass_utils import run_bass_kernel_spmd
import bass_rust

F32 = mybir.dt.float32
BF16 = mybir.dt.bfloat16
FP8 = mybir.dt.float8e4
AX = mybir.AxisListType.X
OP = mybir.AluOpType
ACTF = mybir.ActivationFunctionType
DROW = mybir.MatmulPerfMode.DoubleRow

C = 256
H = W = 64
N = H * W            # 4096
M = 32               # KAttention proj dim
HID = 1024
NT512 = N // 512     # 8
NT128 = N // 128     # 32
EPS = 1e-6
PADW = 66            # 64 + 1 pad each side

BF = ml_dtypes.bfloat16
F8 = ml_dtypes.float8_e4m3

TAPS = [(dy, dx) for dy in (-1, 0, 1) for dx in (-1, 0, 1)]

_CACHE = {}


def _build(gk: float, gc: float):
    nc = bacc.Bacc("TRN2", target_bir_lowering=False, debug=False)

    d = {}
    def din(name, shape, dt):
        d[name] = nc.dram_tensor(name, list(shape), dt, kind="ExternalInput").ap()
    din("xb", (C, N), F32)
    din("yb", (C, N), F32)
    din("xb16", (C, N), BF16)
    din("yb16", (C, N), BF16)
    din("pwT", (C, C), BF16)          # conv pw weight, transposed [cin, cout]
    din("dwd8", (128, 36, 128), FP8)  # diag dw [p, ((mh*9+tap)*2+hl), m]
    din("zbc", (128, 2), F32)         # z1 bias as per-partition cols
    din("tv8", (128, 2, 512), FP8)    # [I256 | v_w.T], kt-subtile layout
    din("kT", (C, M), BF16)
    din("qT8", (128, 2, M), FP8)
    din("w1h8", (128, 2, HID), FP8)   # fc1 weights hi, kt-subtile layout
    din("w1l8", (128, 2, HID), FP8)   # fc1 weights lo
    din("b1c", (128, 8), F32)         # fc1 bias cols per hm block
    din("w2h8", (128, 8, C), FP8)     # fc2 weights hi, k-subtile layout
    din("w2l8", (128, 8, C), FP8)     # fc2 weights lo
    din("b2c", (128, 2), F32)         # fc2 bias cols per mh half
    din("i128f", (128, 128), F32)
    din("i128b", (128, 128), BF16)
    din("o128f8", (128, 1), FP8)
    din("ones32", (1, M), BF16)
    x3o = nc.dram_tensor("x3o", [C, N], F32, kind="ExternalOutput").ap()
    y3o = nc.dram_tensor("y3o", [C, N], F32, kind="ExternalOutput").ap()

    with tile.TileContext(nc) as tc:
        with tc.tile_pool(name="wp", bufs=1) as wp, \
             tc.tile_pool(name="tvc", bufs=6) as tvcp, \
             tc.tile_pool(name="tio", bufs=3) as tio, \
             tc.tile_pool(name="tms", bufs=2) as tms, \
             tc.tile_pool(name="hp", bufs=10) as hp, \
             tc.tile_pool(name="psA", bufs=4, space="PSUM") as psA, \
             tc.tile_pool(name="psL", bufs=2, space="PSUM") as psL, \
             tc.tile_pool(name="psR", bufs=2, space="PSUM") as psR:

            # ---- stream-x input load first (startup long pole), chunked.
            zbf = [[wp.tile([128, N // 4], BF16, tag=f"zbf{h}{q}",
                            name=f"zbf{h}{q}") for q in range(4)]
                   for h in range(2)]

            def load_chunk(z_dram, q, dma_eng, cv_eng=None):
                for h in range(2):
                    sl = slice(q * (N // 4), (q + 1) * (N // 4))
                    dma_eng.dma_start(out=zbf[h][q][:],
                                      in_=z_dram[h*128:(h+1)*128, sl])

            def load_input(z_dram, eng=None):
                qengs = [nc.scalar, nc.gpsimd, nc.sync]
                for q in range(4):
                    load_chunk(z_dram, q, qengs[q % len(qengs)])

            def wtile(name, shape, dt, src):
                t = wp.tile(list(shape), dt, tag=name, name=name)
                nc.sync.dma_start(out=t[:], in_=src)
                return t

            pw_sb = [wtile(f"pw{k}", (128, C), BF16, d["pwT"][k*128:(k+1)*128, :])
                     for k in range(2)]

            load_input(d["xb16"])

            # ---- remaining weights ----
            dwd_sb = wtile("dwd8", (128, 36, 128), FP8, d["dwd8"][:, :, :])
            zbc_sb = wtile("zbc", (128, 2), F32, d["zbc"][:, :])
            tv8_sb = wtile("tv8", (128, 2, 512), FP8, d["tv8"][:, :, :])
            kT_sb = [wtile(f"kT{k}", (128, M), BF16, d["kT"][k*128:(k+1)*128, :])
                     for k in range(2)]
            qT8_sb = wtile("qT8", (128, 2, M), FP8, d["qT8"][:, :, :])
            w1h_sb = wtile("w1h8", (128, 2, HID), FP8, d["w1h8"][:, :, :])
            w1l_sb = wtile("w1l8", (128, 2, HID), FP8, d["w1l8"][:, :, :])
            b1c_sb = wtile("b1c", (128, 8), F32, d["b1c"][:, :])
            w2h_sb = wtile("w2h8", (128, 8, C), FP8, d["w2h8"][:, :, :])
            w2l_sb = wtile("w2l8", (128, 8, C), FP8, d["w2l8"][:, :, :])
            b2c_sb = wtile("b2c", (128, 2), F32, d["b2c"][:, :])
            idf_sb = wtile("i128f", (128, 128), F32, d["i128f"][:, :])
            idb_sb = wtile("i128b", (128, 128), BF16, d["i128b"][:, :])
            o128_sb = wtile("o128f8", (128, 1), FP8, d["o128f8"][:, :])
            o32_sb = wtile("ones32", (1, M), BF16, d["ones32"][:, :])

            # ---- persistent tensors ----
            ppad = [wp.tile([128, PADW, PADW], FP8, tag=f"ppad{h}",
                            name=f"ppad{h}") for h in range(2)]
            z1 = {s: [wp.tile([128, N], BF16, tag=f"z1{s}{h}", name=f"z1{s}{h}")
                      for h in range(2)] for s in range(2)}
            z8 = [[wp.tile([128, 2, N // 4], FP8, tag=f"z8{s}c{c}",
                           name=f"z8{s}c{c}") for c in range(4)]
                  for s in range(2)]

            def z8ap(s, lo, size):
                c, off = divmod(lo, N // 4)
                assert off + size <= N // 4
                return z8[s][c][:, :, off:off+size]
            u8 = [[wp.tile([128, 2, 512], FP8, tag=f"u8{s}n{nt}",
                           name=f"u8{s}n{nt}") for nt in range(NT512)]
                  for s in range(2)]
            ax = [wp.tile([128, N], BF16, tag=f"ax{h}", name=f"ax{h}")
                  for h in range(2)]
            KT = [[wp.tile([128, 8, M], BF16, tag=f"KT{s}g{g}",
                           name=f"KT{s}g{g}") for g in range(4)]
                  for s in range(2)]                       # exp intermediate
            KT8 = [[wp.tile([128, 8, M], FP8, tag=f"KT8{s}g{g}",
                            name=f"KT8{s}g{g}") for g in range(4)]
                   for s in range(2)]                      # softplus, fp8
            Q = [[wp.tile([M, N // 4], BF16, tag=f"Q{s}g{g}",
                          name=f"Q{s}g{g}") for g in range(4)]
                 for s in range(2)]

            def Qap(s, lo, size):
                g, off = divmod(lo, N // 4)
                assert off + size <= N // 4
                return Q[s][g][:, off:off+size]
            invFM_t = wp.tile([1, N], BF16, tag="invFM", name="invFM")
            invFM = [invFM_t, invFM_t]

            # zero pad cells of ppad once (interior overwritten each stream)
            for h in range(2):
                p = ppad[h]
                nc.gpsimd.memset(p[:, 0, :], 0.0)
                nc.gpsimd.memset(p[:, PADW - 1, :], 0.0)
                nc.gpsimd.memset(p[:, :, 0], 0.0)
                nc.gpsimd.memset(p[:, :, PADW - 1], 0.0)

            def dw_rhs(mh, nt, dy, dx):
                """4D moving AP over ppad fp8: [128, 2(bcast), 8, 64] window
                at tap (dy,dx) for output row block nt (8 rows)."""
                a = ppad[mh][:]
                off = a.offset + (1 + nt * 8 + dy) * PADW + (1 + dx)
                return bass_rust.AP(a.tensor, off,
                                    [[PADW * PADW, 128], [0, 2], [PADW, 8],
                                     [1, 64]])

            def pw_unit(mh, nt, alt_copy=False):
                ps = psA.tile([128, 512], F32, tag="mm")
                q, r = divmod(nt, 2)
                for kt in range(2):
                    nc.tensor.matmul(
                        ps[:], pw_sb[kt][:, mh*128:(mh+1)*128],
                        zbf[kt][q][:, r*512:(r+1)*512],
                        start=(kt == 0), stop=(kt == 1))
                h0 = nt * 8
                dst = ppad[mh][:, 1+h0:1+h0+8, 1:65]
                src = ps[:].rearrange("p (h w) -> p h w", h=8)
                if alt_copy and mh == 1:
                    nc.vector.tensor_copy(dst, src)
                else:
                    nc.scalar.copy(out=dst, in_=src)

            def dw_unit(s, mh, nt):
                """depthwise 3x3: 9 fp8 DROW passes, weight hi/lo subtile
                pairs, moving = broadcast-subtile window views of ppad."""
                ps = psA.tile([128, 512], F32, tag="mm")
                for t, (dy, dx) in enumerate(TAPS):
                    st = dwd_sb[:, (mh*9 + t)*2:(mh*9 + t)*2 + 2, :]
                    nc.tensor.matmul(
                        ps[:], st, dw_rhs(mh, nt, dy, dx),
                        start=(t == 0), stop=(t == 8), perf_mode=DROW)
                sl = slice(nt*512, (nt+1)*512)
                if s == 0:
                    nc.vector.tensor_scalar(
                        z1[s][mh][:, sl], ps[:], zbc_sb[:, mh:mh+1], None,
                        OP.add)
                else:
                    nc.scalar.activation(
                        out=z1[s][mh][:, sl], in_=ps[:], func=ACTF.Identity,
                        bias=zbc_sb[:, mh:mh+1])
                # Pool cannot read PSUM: derive z8 from z1 (SBUF->SBUF)
                c, off = divmod(nt*512, N // 4)
                nc.gpsimd.tensor_copy(z8[s][c][:, mh, off:off+512],
                                      z1[s][mh][:, sl])

            def q_proj(s, qt):
                pq = psA.tile([M, 512], F32, tag="mm")
                nc.tensor.matmul(pq[:], qT8_sb[:], z8ap(s, qt*512, 512),
                                 start=True, stop=True, perf_mode=DROW)
                nc.scalar.activation(out=Qap(s, qt*512, 512),
                                     in_=pq[:], func=ACTF.Exp)

            def k_group(s, g):
                pk = psR.tile([128, 4, M], F32, tag="rt")
                for j in range(4):
                    nt = 4*g + j
                    for kt in range(2):
                        nc.tensor.matmul(pk[:, j, :],
                                         z1[s][kt][:, nt*128:(nt+1)*128],
                                         kT_sb[kt][:],
                                         start=(kt == 0), stop=(kt == 1))
                nc.scalar.activation(
                    out=KT[s][4*g // 8][:, (4*g) % 8:(4*g) % 8 + 4, :],
                    in_=pk[:], func=ACTF.Exp)

            def sepconv_x():
                for nt in range(NT512):
                    for mh in range(2):
                        pw_unit(mh, nt)
                    if nt % 2 == 1:
                        q = nt // 2
                        load_chunk(d["yb16"], q, nc.sync)
                    if nt >= 1:
                        for mh in range(2):
                            dw_unit(0, mh, nt - 1)
                    if nt >= 2:
                        q_proj(0, nt - 2)
                        k_group(0, nt - 2)
                for mh in range(2):
                    dw_unit(0, mh, NT512 - 1)
                for qt in range(NT512 - 2, NT512):
                    q_proj(0, qt)
                    k_group(0, qt)

            def sepconv_dw1():
                for nt in range(NT512):
                    for mh in range(2):
                        dw_unit(1, mh, nt)
                    if nt >= 1:
                        q_proj(1, nt - 1)
                        k_group(1, nt - 1)
                q_proj(1, NT512 - 1)
                k_group(1, NT512 - 1)

            def light_a(s, st, filler=None, prefill=None):
                if prefill is not None:
                    prefill()
                with tc.tile_critical():
                    for g in range(4):
                        nc.scalar.activation(
                            out=KT8[s][g][:], in_=KT[s][g][:],
                            func=ACTF.Ln, bias=1.0)
                    for g in range(4):
                        nc.scalar.activation(
                            out=Q[s][g][:], in_=Q[s][g][:],
                            func=ACTF.Ln, bias=1.0)
                pe = [psL.tile([128, C], F32, tag="lv", name=f"pe{i}")
                      for i in range(2)]
                pkv = psR.tile([M, C], F32, tag="rt")
                for pr in range(16):
                    if filler is not None:
                        filler(pr)
                    chp = tvcp.tile([128, 2, 512], FP8, tag="tvc")
                    for j in range(2):
                        nt = 2*pr + j
                        ps = psA.tile([128, 512], F32, tag="mm")
                        nc.tensor.matmul(ps[:], z8ap(s, nt*128, 128),
                                         tv8_sb[:], start=True, stop=True,
                                         perf_mode=DROW)
                        if j == 0:
                            nc.vector.tensor_copy(chp[:, j, :], ps[:])
                        else:
                            nc.scalar.copy(out=chp[:, j, :], in_=ps[:])
                    for mh in range(2):
                        nc.tensor.matmul(pe[mh][:],
                                         chp[:, :, mh*128:(mh+1)*128],
                                         chp[:, :, 0:256],
                                         start=(pr == 0), stop=(pr == 15),
                                         perf_mode=DROW)
                    nc.tensor.matmul(
                        pkv[:],
                        KT8[s][(2*pr) // 8][:, (2*pr) % 8:(2*pr) % 8 + 2, :],
                        chp[:, :, 256:512],
                        start=(pr == 0), stop=(pr == 15),
                        perf_mode=DROW)
                st["pe"], st["pkv"] = pe, pkv

            def light_chain(s, st):
                pe, pkv = st["pe"], st["pkv"]
                kv = tms.tile([M, C], BF16, tag="kv", bufs=2)
                nc.vector.tensor_scalar(kv[:], pkv[:], gk, None, OP.mult)
                st["kv"] = kv
                pks = psR.tile([M, 1], F32, tag="rt")
                for nt in range(NT128):
                    nc.tensor.matmul(pks[:],
                                     KT8[s][nt // 8][:, nt % 8, :],
                                     o128_sb[:],
                                     start=(nt == 0), stop=(nt == NT128 - 1))
                ksum = tms.tile([M, 1], BF16, tag="ksum")
                nc.vector.tensor_scalar(ksum[:], pks[:], EPS, None, OP.add)
                pdT = psR.tile([128, NT128], F32, tag="rt")
                for t in range(NT128):
                    nc.tensor.matmul(pdT[:, t:t+1], Qap(s, t*128, 128),
                                     ksum[:], start=True, stop=True)
                invA = tms.tile([128, NT128], F32, tag="invA")
                nc.vector.reciprocal(invA[:], pdT[:])
                piT = psR.tile([M, 128], F32, tag="rt")
                nc.tensor.transpose(piT[:], invA[:], idf_sb[:])
                invT = tms.tile([NT128, 128], BF16, tag="invT")
                nc.vector.tensor_copy(invT[:], piT[:])
                nc.gpsimd.dma_start(out=invFM[s][0:1, :], in_=invT[:])
                att = [tms.tile([128, C], BF16, tag=f"att{mh}", name=f"att{mh}")
                       for mh in range(2)]
                for mh in range(2):
                    rmax = tms.tile([128, 1], F32, tag="rmax")
                    nc.vector.tensor_reduce(rmax[:], pe[mh][:], axis=AX,
                                            op=OP.min)
                    tdiff = tms.tile([128, C], F32, tag="tdiff")
                    nc.vector.tensor_scalar(tdiff[:], pe[mh][:], rmax[:], 30.0,
                                            OP.subtract, OP.min)
                    ex = tms.tile([128, C], F32, tag="ex")
                    nc.scalar.activation(out=ex[:], in_=tdiff[:], func=ACTF.Exp,
                                         scale=-1.0)
                    rsum = tms.tile([128, 1], F32, tag="rsum")
                    nc.vector.reduce_sum(rsum[:], ex[:], axis=AX)
                    rinv = tms.tile([128, 1], F32, tag="rinv")
                    nc.vector.reciprocal(rinv[:], rsum[:])
                    nc.vector.tensor_scalar(att[mh][:], ex[:], rinv[:], gc,
                                            OP.mult, OP.mult)
                att8T = tms.tile([128, 2, C], FP8, tag="att8T", bufs=2)
                for mh in range(2):
                    for kt in range(2):
                        pat = psR.tile([128, 128], BF16, tag="rt")
                        nc.tensor.transpose(pat[:],
                                            att[mh][:, kt*128:(kt+1)*128],
                                            idb_sb[:])
                        nc.vector.tensor_copy(
                            att8T[:, kt, mh*128:(mh+1)*128], pat[:])
                st["att8T"] = att8T

            def light_post_piece(s, nt):
                pib = psA.tile([M, 512], F32, tag="mm")
                nc.tensor.matmul(pib[:], o32_sb[0:1, :],
                                 invFM[s][0:1, nt*512:(nt+1)*512],
                                 start=True, stop=True)
                nc.vector.tensor_tensor(Qap(s, nt*512, 512),
                                        Qap(s, nt*512, 512), pib[:],
                                        OP.mult)

            def light_chain_post(s, st):
                for nt in range(NT512):
                    light_post_piece(s, nt)

            def light_b(s, st, nts):
                """wv + catt in one PSUM; combine a = 2*z1 + psum.
                s=0: store into ax.  s=1: ay consumed on the fly ->
                u8[0] = fp8(x1*ay), u8[1] = fp8(y1*ax)."""
                z = z1[s]
                kv, att8T = st["kv"], st["att8T"]
                for nt in nts:
                    pos = []
                    for mh in range(2):
                        po = psA.tile([128, 512], F32, tag="mm")
                        nc.tensor.matmul(po[:], kv[:, mh*128:(mh+1)*128],
                                         Qap(s, nt*512, 512),
                                         start=True, stop=False)
                        nc.tensor.matmul(po[:],
                                         att8T[:, :, mh*128:(mh+1)*128],
                                         z8ap(s, nt*512, 512),
                                         start=False, stop=True,
                                         perf_mode=DROW)
                        pos.append(po)
                    for mh in range(2):
                        sl = slice(nt*512, (nt+1)*512)
                        if s == 0:
                            nc.vector.scalar_tensor_tensor(
                                ax[mh][:, sl], z[mh][:, sl], 2.0, pos[mh][:],
                                OP.mult, OP.add)
                            # u_y = y1*ax ready as soon as ax lands (Pool,
                            # idle in this phase) -> mlp-Y can run early
                            nc.gpsimd.tensor_tensor(u8[1][nt][:, mh, :],
                                                    z1[1][mh][:, sl],
                                                    ax[mh][:, sl], OP.mult)
                        else:
                            ay_t = tvcp.tile([128, 512], BF16, tag="ayt")
                            nc.vector.scalar_tensor_tensor(
                                ay_t[:], z[mh][:, sl], 2.0, pos[mh][:],
                                OP.mult, OP.add)
                            nc.gpsimd.tensor_tensor(u8[0][nt][:, mh, :],
                                                    z1[0][mh][:, sl], ay_t[:],
                                                    OP.mult)

            # mlp pieces -----------------------------------------------------
            def mlp_fc1(s, nt, min6_dve=3, relu_dve=0):
                """hidden = relu6(fc1(u8) + b): 2 DROW passes per 128-block
                (w1 hi + lo); relu+bias drains on Act (last `relu_dve`
                halves on DVE), min6 split DVE/Pool."""
                hps = []
                for p in range(4):
                    h8 = hp.tile([128, 2, 512], FP8, tag="h")
                    for i in range(2):
                        hm = 2*p + i
                        ph = psA.tile([128, 512], F32, tag="mm")
                        for wsb in (w1h_sb, w1l_sb):
                            nc.tensor.matmul(
                                ph[:], wsb[:, :, hm*128:(hm+1)*128],
                                u8[s][nt][:],
                                start=(wsb is w1h_sb), stop=(wsb is w1l_sb),
                                perf_mode=DROW)
                        if 2*p + i >= 8 - relu_dve:
                            nc.vector.tensor_scalar(
                                h8[:, i, :], ph[:], b1c_sb[:, hm:hm+1], 0.0,
                                OP.add, OP.max)
                        else:
                            nc.scalar.activation(out=h8[:, i, :], in_=ph[:],
                                                 func=ACTF.Relu,
                                                 bias=b1c_sb[:, hm:hm+1])
                    if p < min6_dve:
                        nc.vector.tensor_scalar(h8[:], h8[:], 6.0, None,
                                                OP.min)
                    else:
                        nc.gpsimd.tensor_scalar(h8[:], h8[:], 6.0, None,
                                                OP.min)
                    hps.append(h8)
                return hps

            def mlp_fc2(hps):
                outs = []
                for mh in range(2):
                    p2 = psL.tile([128, 512], F32, tag="lv")
                    for p in range(4):
                        for k, wsb in enumerate((w2h_sb, w2l_sb)):
                            nc.tensor.matmul(
                                p2[:], wsb[:, 2*p:2*p+2, mh*128:(mh+1)*128],
                                hps[p][:], start=(p == 0 and k == 0),
                                stop=(p == 3 and k == 1),
                                perf_mode=DROW)
                    outs.append(p2)
                return outs

            def mlp_y_unit(nt):
                """y2 = mlp(u_y); y3 = y + y2 -> DRAM (re-read in the x loop,
                same sync queue so the round trip is ordered)."""
                hs_y = mlp_fc1(1, nt, min6_dve=2, relu_dve=0)
                y2p = mlp_fc2(hs_y)
                for mh in range(2):
                    yin = tio.tile([128, 512], F32, tag="zin", bufs=6)
                    nc.sync.dma_start(out=yin[:],
                                      in_=d["yb"][mh*128:(mh+1)*128,
                                                  nt*512:(nt+1)*512])
                    y3t = tio.tile([128, 512], F32, tag="y3t", bufs=4)
                    nc.vector.scalar_tensor_tensor(
                        y3t[:], y2p[mh][:], b2c_sb[:, mh:mh+1], yin[:],
                        OP.add, OP.add)
                    nc.sync.dma_start(out=y3o[mh*128:(mh+1)*128,
                                              nt*512:(nt+1)*512], in_=y3t[:])

            st0, st1 = {}, {}
            sepconv_x()
            light_a(0, st0,
                    filler=lambda u: (pw_unit((u // 2) // 4,
                                              4 + (u // 2) % 4, alt_copy=True)
                                      if u % 2 == 0 else None),
                    prefill=lambda: [pw_unit(mh, nt, alt_copy=True)
                                     for mh in range(2) for nt in range(4)])
            light_chain(0, st0)
            sepconv_dw1()              # PE filler for chain(0) latency
            light_chain_post(0, st0)
            light_a(1, st1,
                    filler=lambda u: (light_b(0, st0, [u // 4 + 3])
                                      if u % 4 == 0 and u < 12 else None),
                    prefill=lambda: light_b(0, st0, range(0, 3)))
            light_b(0, st0, range(6, NT512))
            light_chain(1, st1)
            for nt in range(NT512):    # mlp-Y fills chain(1) latency
                mlp_y_unit(nt)
                light_post_piece(1, nt)

            # x-stream mlp + final adds, tile by tile
            for step in range(NT512 + 1):
                if step < NT512:
                    light_b(1, st1, [step])
                if step == 0:
                    continue
                nt = step - 1
                ins = []
                for mh in range(2):
                    y3in = tio.tile([128, 512], F32, tag="zin", bufs=6)
                    nc.sync.dma_start(out=y3in[:],
                                      in_=y3o[mh*128:(mh+1)*128,
                                              nt*512:(nt+1)*512])
                    xin = tio.tile([128, 512], F32, tag="zin", bufs=6)
                    nc.sync.dma_start(out=xin[:],
                                      in_=d["xb"][mh*128:(mh+1)*128,
                                                  nt*512:(nt+1)*512])
                    ins.append((y3in, xin))
                hs_x = mlp_fc1(0, nt, min6_dve=4)
                x2p = mlp_fc2(hs_x)
                for mh in range(2):
                    y3in, xin = ins[mh]
                    x3t = tio.tile([128, 512], F32, tag="x3t")
                    nc.vector.scalar_tensor_tensor(
                        x3t[:], x2p[mh][:], b2c_sb[:, mh:mh+1], y3in[:],
                        OP.add, OP.add)
                    nc.gpsimd.tensor_tensor(x3t[:], x3t[:], xin[:], OP.add)
                    nc.sync.dma_start(out=x3o[mh*128:(mh+1)*128,
                                              nt*512:(nt+1)*512], in_=x3t[:])
    nc.compile()
    return nc


def _hilo(a):
    hi = np.clip(a, -240.0, 240.0).astype(F8)
    lo = (a - hi.astype(np.float32)).astype(F8)
    return hi, lo


def _prep_weights(i):
    """Host-side weight folding; returns dict of DMA-ready arrays."""
    bf = lambda a: np.ascontiguousarray(a).astype(BF)
    f8 = lambda a: np.ascontiguousarray(a).astype(F8)
    f32 = lambda a: np.ascontiguousarray(a, dtype=np.float32)

    pw = f32(i["conv_pw_w"])            # [C, C]  out,in
    dw = f32(i["conv_dw_w"])[:, 0]      # [C, 3, 3]
    cs, ct = f32(i["conv_bn_s"]), f32(i["conv_bn_t"])
    ns, nt_ = f32(i["norm_s"]), f32(i["norm_t"])
    dwf = dw * (cs * ns)[:, None, None]                    # fold BN+norm scale
    zbias = ct * ns + nt_                                  # fold BN+norm shift
    dwd = np.zeros((128, 36, 128), np.float32)
    for half in range(2):
        for tap, (dy, dx) in enumerate(TAPS):
            w = dwf[half*128:(half+1)*128, dy+1, dx+1]
            whi, wlo = _hilo(w)
            base = (half*9 + tap) * 2
            dwd[np.arange(128), base, np.arange(128)] = whi.astype(np.float32)
            dwd[np.arange(128), base+1, np.arange(128)] = wlo.astype(np.float32)
    tv = np.concatenate([np.eye(C, dtype=np.float32), f32(i["v_w"]).T], axis=1)
    tv8 = tv.reshape(2, 128, 512).transpose(1, 0, 2)       # [128, kt, 512]
    s1, t1 = f32(i["bn1_s"]), f32(i["bn1_t"])
    s2, t2 = f32(i["bn2_s"]), f32(i["bn2_t"])
    fc1, fb1 = f32(i["fc1_w"]), f32(i["fc1_b"])
    fc2, fb2 = f32(i["fc2_w"]), f32(i["fc2_b"])
    w1f = (s1[:, None] * fc1) * ns[None, :]
    b1f = s1 * (fc1 @ nt_ + fb1) + t1
    w2f = s2[:, None] * fc2
    b2f = s2 * fb2 + t2

    w1T = w1f.T.reshape(2, 128, HID).transpose(1, 0, 2)    # [128, kt, HID]
    w1h, w1l = _hilo(w1T)
    w2T = w2f.T.reshape(8, 128, C).transpose(1, 0, 2)      # [128, k8, C]
    w2h, w2l = _hilo(w2T)
    qT8 = f32(i["q_w"]).T.reshape(2, 128, M).transpose(1, 0, 2)

    return {
        "pwT": bf(pw.T),
        "dwd8": f8(dwd),
        "zbc": f32(zbias.reshape(2, 128).T),
        "tv8": f8(tv8),
        "kT": bf(f32(i["k_w"]).T),
        "qT8": f8(qT8),
        "w1h8": w1h, "w1l8": w1l,
        "b1c": f32(b1f.reshape(8, 128).T),
        "w2h8": w2h, "w2l8": w2l,
        "b2c": f32(b2f.reshape(2, 128).T),
        "i128f": np.eye(128, dtype=np.float32),
        "i128b": bf(np.eye(128)),
        "o128f8": f8(np.ones((128, 1))),
        "ones32": bf(np.ones((1, M))),
    }


def kernel(**inputs):
    x = np.ascontiguousarray(inputs["x"], dtype=np.float32)
    y = np.ascontiguousarray(inputs["y"], dtype=np.float32)
    B = x.shape[0]
    gk = float(np.asarray(inputs["gamma_k"]).reshape(-1)[0])
    gc = float(np.asarray(inputs["gamma_c"]).reshape(-1)[0])

    wmaps = _prep_weights(inputs)
    key = (gk, gc) + tuple(hash(v.tobytes()) for _, v in sorted(wmaps.items()))
    if key not in _CACHE:
        _CACHE.clear()
        _CACHE[key] = _build(gk, gc)
    nc = _CACHE[key]

    in_maps = []
    for b in range(B):
        m = dict(wmaps)
        m["xb"] = x[b].reshape(C, N)
        m["yb"] = y[b].reshape(C, N)
        m["xb16"] = x[b].reshape(C, N).astype(BF)
        m["yb16"] = y[b].reshape(C, N).astype(BF)
        in_maps.append(m)

    res = run_bass_kernel_spmd(nc, in_maps, list(range(B)))
    x3 = np.stack([res.results[b]["x3o"].reshape(C, H, W) for b in range(B)])
    y3 = np.stack([res.results[b]["y3o"].reshape(C, H, W) for b in range(B)])
    return (x3.astype(np.float32), y3.astype(np.float32))


# revision 9
# speedup vs baseline: 1.0554x; 1.0123x over previous
"""Trainium2 Bass kernel for nn_Corssattention (dense_transformer), v2.

Full inputs in, full outputs out. Sharding: data-parallel over batch B=8,
one sample per NeuronCore, weights replicated, no cross-core comm.

v2 over baseline: the three big bf16 matmul groups move to fp8 DoubleRow
with host-side hi/lo weight splitting (whi=f8(w), wlo=f8(w-whi); a DROW
pass per term keeps ~10-bit effective weight precision at 0.5 cyc/row):
  - dw 3x3: 9 DROW passes (tap hi/lo pairs; moving AP = fp8 padded image
    window with a stride-0 subtile broadcast dim) instead of 9 bf16 diags
  - fc1: u in fp8, w1 hi/lo -> 2 DROW passes instead of 2 bf16 passes
  - fc2: w2 hi/lo -> 8 DROW passes (bf16-grade weights, was single fp8)
  - q-proj from z8 via DROW
xin/yin final-residual DMAs ride the SP queue (HWDGE) instead of Pool
SWDGE; drains rebalanced across Act/DVE/Pool.
"""

import numpy as np
import ml_dtypes

import concourse.bass as bass
from concourse import bacc
import concourse.mybir as mybir
import concourse.tile as tile
from concourse.bass_utils import run_bass_kernel_spmd
import bass_rust

F32 = mybir.dt.float32
BF16 = mybir.dt.bfloat16
FP8 = mybir.dt.float8e4
AX = mybir.AxisListType.X
OP = mybir.AluOpType
ACTF = mybir.ActivationFunctionType
DROW = mybir.MatmulPerfMode.DoubleRow

C = 256
H = W = 64
N = H * W            # 4096
M = 32               # KAttention proj dim
HID = 1024
NT512 = N // 512     # 8
NT128 = N // 128     # 32
EPS = 1e-6
PADW = 66            # 64 + 1 pad each side

BF = ml_dtypes.bfloat16
F8 = ml_dtypes.float8_e4m3

TAPS = [(dy, dx) for dy in (-1, 0, 1) for dx in (-1, 0, 1)]

_CACHE = {}


def _build(gk: float, gc: float):
    nc = bacc.Bacc("TRN2", target_bir_lowering=False, debug=False)

    d = {}
    def din(name, shape, dt):
        d[name] = nc.dram_tensor(name, list(shape), dt, kind="ExternalInput").ap()
    din("xb", (C, N), F32)
    din("yb", (C, N), F32)
    din("xb16", (C, N), BF16)
    din("yb16", (C, N), BF16)
    din("pwT", (C, C), BF16)          # conv pw weight, transposed [cin, cout]
    din("dwd8", (128, 36, 128), FP8)  # diag dw [p, ((mh*9+tap)*2+hl), m]
    din("zbc", (128, 2), F32)         # z1 bias as per-partition cols
    din("tv8", (128, 2, 512), FP8)    # [I256 | v_w.T], kt-subtile layout
    din("kT", (C, M), BF16)
    din("qT8", (128, 2, M), FP8)
    din("w1h8", (128, 2, HID), FP8)   # fc1 weights hi, kt-subtile layout
    din("w1l8", (128, 2, HID), FP8)   # fc1 weights lo
    din("b1c", (128, 8), F32)         # fc1 bias cols per hm block
    din("w2h8", (128, 8, C), FP8)     # fc2 weights hi, k-subtile layout
    din("w2l8", (128, 8, C), FP8)     # fc2 weights lo
    din("b2c", (128, 2), F32)         # fc2 bias cols per mh half
    din("i128f", (128, 128), F32)
    din("i128b", (128, 128), BF16)
    din("o128f8", (128, 1), FP8)
    din("ones32", (1, M), BF16)
    x3o = nc.dram_tensor("x3o", [C, N], F32, kind="ExternalOutput").ap()
    y3o = nc.dram_tensor("y3o", [C, N], F32, kind="ExternalOutput").ap()

    with tile.TileContext(nc) as tc:
        with tc.tile_pool(name="wp", bufs=1) as wp, \
             tc.tile_pool(name="tvc", bufs=6) as tvcp, \
             tc.tile_pool(name="tio", bufs=3) as tio, \
             tc.tile_pool(name="tms", bufs=2) as tms, \
             tc.tile_pool(name="hp", bufs=10) as hp, \
             tc.tile_pool(name="psA", bufs=4, space="PSUM") as psA, \
             tc.tile_pool(name="psL", bufs=2, space="PSUM") as psL, \
             tc.tile_pool(name="psR", bufs=2, space="PSUM") as psR:

            # ---- stream-x input load first (startup long pole), chunked.
            zbf = [[wp.tile([128, N // 4], BF16, tag=f"zbf{h}{q}",
                            name=f"zbf{h}{q}") for q in range(4)]
                   for h in range(2)]

            def load_chunk(z_dram, q, dma_eng, cv_eng=None):
                for h in range(2):
                    sl = slice(q * (N // 4), (q + 1) * (N // 4))
                    dma_eng.dma_start(out=zbf[h][q][:],
                                      in_=z_dram[h*128:(h+1)*128, sl])

            def load_input(z_dram, eng=None):
                qengs = [nc.scalar, nc.gpsimd, nc.sync]
                for q in range(4):
                    load_chunk(z_dram, q, qengs[q % len(qengs)])

            def wtile(name, shape, dt, src):
                t = wp.tile(list(shape), dt, tag=name, name=name)
                nc.sync.dma_start(out=t[:], in_=src)
                return t

            pw_sb = [wtile(f"pw{k}", (128, C), BF16, d["pwT"][k*128:(k+1)*128, :])
                     for k in range(2)]

            load_input(d["xb16"])

            # ---- remaining weights ----
            dwd_sb = wtile("dwd8", (128, 36, 128), FP8, d["dwd8"][:, :, :])
            zbc_sb = wtile("zbc", (128, 2), F32, d["zbc"][:, :])
            tv8_sb = wtile("tv8", (128, 2, 512), FP8, d["tv8"][:, :, :])
            kT_sb = [wtile(f"kT{k}", (128, M), BF16, d["kT"][k*128:(k+1)*128, :])
                     for k in range(2)]
            qT8_sb = wtile("qT8", (128, 2, M), FP8, d["qT8"][:, :, :])
            w1h_sb = wtile("w1h8", (128, 2, HID), FP8, d["w1h8"][:, :, :])
            w1l_sb = wtile("w1l8", (128, 2, HID), FP8, d["w1l8"][:, :, :])
            b1c_sb = wtile("b1c", (128, 8), F32, d["b1c"][:, :])
            w2h_sb = wtile("w2h8", (128, 8, C), FP8, d["w2h8"][:, :, :])
            w2l_sb = wtile("w2l8", (128, 8, C), FP8, d["w2l8"][:, :, :])
            b2c_sb = wtile("b2c", (128, 2), F32, d["b2c"][:, :])
            idf_sb = wtile("i128f", (128, 128), F32, d["i128f"][:, :])
            idb_sb = wtile("i128b", (128, 128), BF16, d["i128b"][:, :])
            o128_sb = wtile("o128f8", (128, 1), FP8, d["o128f8"][:, :])
            o32_sb = wtile("ones32", (1, M), BF16, d["ones32"][:, :])

            # ---- persistent tensors ----
            ppad = [wp.tile([128, PADW, PADW], FP8, tag=f"ppad{h}",
                            name=f"ppad{h}") for h in range(2)]
            z1 = {s: [wp.tile([128, N], BF16, tag=f"z1{s}{h}", name=f"z1{s}{h}")
                      for h in range(2)] for s in range(2)}
            z8 = [[wp.tile([128, 2, N // 4], FP8, tag=f"z8{s}c{c}",
                           name=f"z8{s}c{c}") for c in range(4)]
                  for s in range(2)]

            def z8ap(s, lo, size):
                c, off = divmod(lo, N // 4)
                assert off + size <= N // 4
                return z8[s][c][:, :, off:off+size]
            u8 = [[wp.tile([128, 2, 512], FP8, tag=f"u8{s}n{nt}",
                           name=f"u8{s}n{nt}") for nt in range(NT512)]
                  for s in range(2)]
            ax = [wp.tile([128, N], BF16, tag=f"ax{h}", name=f"ax{h}")
                  for h in range(2)]
            KT = [[wp.tile([128, 8, M], BF16, tag=f"KT{s}g{g}",
                           name=f"KT{s}g{g}") for g in range(4)]
                  for s in range(2)]                       # exp intermediate
            KT8 = [[wp.tile([128, 8, M], FP8, tag=f"KT8{s}g{g}",
                            name=f"KT8{s}g{g}") for g in range(4)]
                   for s in range(2)]                      # softplus, fp8
            Q = [[wp.tile([M, N // 4], BF16, tag=f"Q{s}g{g}",
                          name=f"Q{s}g{g}") for g in range(4)]
                 for s in range(2)]

            def Qap(s, lo, size):
                g, off = divmod(lo, N // 4)
                assert off + size <= N // 4
                return Q[s][g][:, off:off+size]
            invFM_t = wp.tile([1, N], BF16, tag="invFM", name="invFM")
            invFM = [invFM_t, invFM_t]

            # zero pad cells of ppad once (interior overwritten each stream)
            for h in range(2):
                p = ppad[h]
                nc.gpsimd.memset(p[:, 0, :], 0.0)
                nc.gpsimd.memset(p[:, PADW - 1, :], 0.0)
                nc.gpsimd.memset(p[:, :, 0], 0.0)
                nc.gpsimd.memset(p[:, :, PADW - 1], 0.0)

            def dw_rhs(mh, nt, dy, dx):
                """4D moving AP over ppad fp8: [128, 2(bcast), 8, 64] window
                at tap (dy,dx) for output row block nt (8 rows)."""
                a = ppad[mh][:]
                off = a.offset + (1 + nt * 8 + dy) * PADW + (1 + dx)
                return bass_rust.AP(a.tensor, off,
                                    [[PADW * PADW, 128], [0, 2], [PADW, 8],
                                     [1, 64]])

            def pw_unit(mh, nt, alt_copy=False):
                ps = psA.tile([128, 512], F32, tag="mm")
                q, r = divmod(nt, 2)
                for kt in range(2):
                    nc.tensor.matmul(
                        ps[:], pw_sb[kt][:, mh*128:(mh+1)*128],
                        zbf[kt][q][:, r*512:(r+1)*512],
                        start=(kt == 0), stop=(kt == 1))
                h0 = nt * 8
                dst = ppad[mh][:, 1+h0:1+h0+8, 1:65]
                src = ps[:].rearrange("p (h w) -> p h w", h=8)
                if alt_copy and mh == 1:
                    nc.vector.tensor_copy(dst, src)
                else:
                    nc.scalar.copy(out=dst, in_=src)

            def dw_unit(s, mh, nt):
                """depthwise 3x3: 9 fp8 DROW passes, weight hi/lo subtile
                pairs, moving = broadcast-subtile window views of ppad."""
                ps = psA.tile([128, 512], F32, tag="mm")
                for t, (dy, dx) in enumerate(TAPS):
                    st = dwd_sb[:, (mh*9 + t)*2:(mh*9 + t)*2 + 2, :]
                    nc.tensor.matmul(
                        ps[:], st, dw_rhs(mh, nt, dy, dx),
                        start=(t == 0), stop=(t == 8), perf_mode=DROW)
                sl = slice(nt*512, (nt+1)*512)
                if s == 0:
                    nc.vector.tensor_scalar(
                        z1[s][mh][:, sl], ps[:], zbc_sb[:, mh:mh+1], None,
                        OP.add)
                else:
                    nc.scalar.activation(
                        out=z1[s][mh][:, sl], in_=ps[:], func=ACTF.Identity,
                        bias=zbc_sb[:, mh:mh+1])
                # Pool cannot read PSUM: derive z8 from z1 (SBUF->SBUF)
                c, off = divmod(nt*512, N // 4)
                nc.gpsimd.tensor_copy(z8[s][c][:, mh, off:off+512],
                                      z1[s][mh][:, sl])

            def q_proj(s, qt):
                pq = psA.tile([M, 512], F32, tag="mm")
                nc.tensor.matmul(pq[:], qT8_sb[:], z8ap(s, qt*512, 512),
                                 start=True, stop=True, perf_mode=DROW)
                nc.scalar.activation(out=Qap(s, qt*512, 512),
                                     in_=pq[:], func=ACTF.Exp)

            def k_group(s, g):
                pk = psR.tile([128, 4, M], F32, tag="rt")
                for j in range(4):
                    nt = 4*g + j
                    for kt in range(2):
                        nc.tensor.matmul(pk[:, j, :],
                                         z1[s][kt][:, nt*128:(nt+1)*128],
                                         kT_sb[kt][:],
                                         start=(kt == 0), stop=(kt == 1))
                nc.scalar.activation(
                    out=KT[s][4*g // 8][:, (4*g) % 8:(4*g) % 8 + 4, :],
                    in_=pk[:], func=ACTF.Exp)

            def sepconv_x():
                for nt in range(NT512):
                    for mh in range(2):
                        pw_unit(mh, nt)
                    if nt % 2 == 1:
                        q = nt // 2
                        load_chunk(d["yb16"], q, nc.sync)
                    if nt >= 1:
                        for mh in range(2):
                            dw_unit(0, mh, nt - 1)
                    if nt >= 2:
                        q_proj(0, nt - 2)
                        k_group(0, nt - 2)
                for mh in range(2):
                    dw_unit(0, mh, NT512 - 1)
                for qt in range(NT512 - 2, NT512):
                    q_proj(0, qt)
                    k_group(0, qt)

            def sepconv_dw1():
                for nt in range(NT512):
                    for mh in range(2):
                        dw_unit(1, mh, nt)
                    if nt >= 1:
                        q_proj(1, nt - 1)
                        k_group(1, nt - 1)
                q_proj(1, NT512 - 1)
                k_group(1, NT512 - 1)

            def light_a(s, st, filler=None, prefill=None):
                if prefill is not None:
                    prefill()
                with tc.tile_critical():
                    for g in range(4):
                        nc.scalar.activation(
                            out=KT8[s][g][:], in_=KT[s][g][:],
                            func=ACTF.Ln, bias=1.0)
                    for g in range(4):
                        nc.scalar.activation(
                            out=Q[s][g][:], in_=Q[s][g][:],
                            func=ACTF.Ln, bias=1.0)
                pe = [psL.tile([128, C], F32, tag="lv", name=f"pe{i}")
                      for i in range(2)]
                pkv = psR.tile([M, C], F32, tag="rt")
                for pr in range(16):
                    if filler is not None:
                        filler(pr)
                    chp = tvcp.tile([128, 2, 512], FP8, tag="tvc")
                    for j in range(2):
                        nt = 2*pr + j
                        ps = psA.tile([128, 512], F32, tag="mm")
                        nc.tensor.matmul(ps[:], z8ap(s, nt*128, 128),
                                         tv8_sb[:], start=True, stop=True,
                                         perf_mode=DROW)
                        if j == 0:
                            nc.vector.tensor_copy(chp[:, j, :], ps[:])
                        else:
                            nc.scalar.copy(out=chp[:, j, :], in_=ps[:])
                    for mh in range(2):
                        nc.tensor.matmul(pe[mh][:],
                                         chp[:, :, mh*128:(mh+1)*128],
                                         chp[:, :, 0:256],
                                         start=(pr == 0), stop=(pr == 15),
                                         perf_mode=DROW)
                    nc.tensor.matmul(
                        pkv[:],
                        KT8[s][(2*pr) // 8][:, (2*pr) % 8:(2*pr) % 8 + 2, :],
                        chp[:, :, 256:512],
                        start=(pr == 0), stop=(pr == 15),
                        perf_mode=DROW)
                st["pe"], st["pkv"] = pe, pkv

            def light_chain(s, st):
                pe, pkv = st["pe"], st["pkv"]
                kv = tms.tile([M, C], BF16, tag="kv", bufs=2)
                nc.vector.tensor_scalar(kv[:], pkv[:], gk, None, OP.mult)
                st["kv"] = kv
                pks = psR.tile([M, 1], F32, tag="rt")
                for nt in range(NT128):
                    nc.tensor.matmul(pks[:],
                                     KT8[s][nt // 8][:, nt % 8, :],
                                     o128_sb[:],
                                     start=(nt == 0), stop=(nt == NT128 - 1))
                ksum = tms.tile([M, 1], BF16, tag="ksum")
                nc.vector.tensor_scalar(ksum[:], pks[:], EPS, None, OP.add)
                pdT = psR.tile([128, NT128], F32, tag="rt")
                for t in range(NT128):
                    nc.tensor.matmul(pdT[:, t:t+1], Qap(s, t*128, 128),
                                     ksum[:], start=True, stop=True)
                invA = tms.tile([128, NT128], F32, tag="invA")
                nc.vector.reciprocal(invA[:], pdT[:])
                piT = psR.tile([M, 128], F32, tag="rt")
                nc.tensor.transpose(piT[:], invA[:], idf_sb[:])
                invT = tms.tile([NT128, 128], BF16, tag="invT")
                nc.vector.tensor_copy(invT[:], piT[:])
                nc.sync.dma_start(out=invFM[s][0:1, :], in_=invT[:])
                att = [tms.tile([128, C], BF16, tag=f"att{mh}", name=f"att{mh}")
                       for mh in range(2)]
                for mh in range(2):
                    rmax = tms.tile([128, 1], F32, tag="rmax")
                    nc.vector.tensor_reduce(rmax[:], pe[mh][:], axis=AX,
                                            op=OP.min)
                    tdiff = tms.tile([128, C], F32, tag="tdiff")
                    nc.vector.tensor_scalar(tdiff[:], pe[mh][:], rmax[:], 30.0,
                                            OP.subtract, OP.min)
                    ex = tms.tile([128, C], F32, tag="ex")
                    nc.scalar.activation(out=ex[:], in_=tdiff[:], func=ACTF.Exp,
                                         scale=-1.0)
                    rsum = tms.tile([128, 1], F32, tag="rsum")
                    nc.vector.reduce_sum(rsum[:], ex[:], axis=AX)
                    rinv = tms.tile([128, 1], F32, tag="rinv")
                    nc.vector.reciprocal(rinv[:], rsum[:])
                    nc.vector.tensor_scalar(att[mh][:], ex[:], rinv[:], gc,
                                            OP.mult, OP.mult)
                att8T = tms.tile([128, 2, C], FP8, tag="att8T", bufs=2)
                for mh in range(2):
                    for kt in range(2):
                        pat = psR.tile([128, 128], BF16, tag="rt")
                        nc.tensor.transpose(pat[:],
                                            att[mh][:, kt*128:(kt+1)*128],
                                            idb_sb[:])
                        nc.vector.tensor_copy(
                            att8T[:, kt, mh*128:(mh+1)*128], pat[:])
                st["att8T"] = att8T

            def light_post_piece(s, nt):
                pib = psA.tile([M, 512], F32, tag="mm")
                nc.tensor.matmul(pib[:], o32_sb[0:1, :],
                                 invFM[s][0:1, nt*512:(nt+1)*512],
                                 start=True, stop=True)
                nc.vector.tensor_tensor(Qap(s, nt*512, 512),
                                        Qap(s, nt*512, 512), pib[:],
                                        OP.mult)

            def light_chain_post(s, st):
                for nt in range(NT512):
                    light_post_piece(s, nt)

            def light_b(s, st, nts):
                """wv + catt in one PSUM; combine a = 2*z1 + psum.
                s=0: store into ax.  s=1: ay consumed on the fly ->
                u8[0] = fp8(x1*ay), u8[1] = fp8(y1*ax)."""
                z = z1[s]
                kv, att8T = st["kv"], st["att8T"]
                for nt in nts:
                    pos = []
                    for mh in range(2):
                        po = psA.tile([128, 512], F32, tag="mm")
                        nc.tensor.matmul(po[:], kv[:, mh*128:(mh+1)*128],
                                         Qap(s, nt*512, 512),
                                         start=True, stop=False)
                        nc.tensor.matmul(po[:],
                                         att8T[:, :, mh*128:(mh+1)*128],
                                         z8ap(s, nt*512, 512),
                                         start=False, stop=True,
                                         perf_mode=DROW)
                        pos.append(po)
                    for mh in range(2):
                        sl = slice(nt*512, (nt+1)*512)
                        if s == 0:
                            nc.vector.scalar_tensor_tensor(
                                ax[mh][:, sl], z[mh][:, sl], 2.0, pos[mh][:],
                                OP.mult, OP.add)
                            # u_y = y1*ax ready as soon as ax lands (Pool,
                            # idle in this phase) -> mlp-Y can run early
                            nc.gpsimd.tensor_tensor(u8[1][nt][:, mh, :],
                                                    z1[1][mh][:, sl],
                                                    ax[mh][:, sl], OP.mult)
                        else:
                            ay_t = tvcp.tile([128, 512], BF16, tag="ayt")
                            nc.vector.scalar_tensor_tensor(
                                ay_t[:], z[mh][:, sl], 2.0, pos[mh][:],
                                OP.mult, OP.add)
                            nc.gpsimd.tensor_tensor(u8[0][nt][:, mh, :],
                                                    z1[0][mh][:, sl], ay_t[:],
                                                    OP.mult)

            # mlp pieces -----------------------------------------------------
            def mlp_fc1(s, nt, min6_dve=3, relu_dve=0):
                """hidden = relu6(fc1(u8) + b): 2 DROW passes per 128-block
                (w1 hi + lo); relu+bias drains on Act (last `relu_dve`
                halves on DVE), min6 split DVE/Pool."""
                hps = []
                for p in range(4):
                    h8 = hp.tile([128, 2, 512], FP8, tag="h")
                    for i in range(2):
                        hm = 2*p + i
                        ph = psA.tile([128, 512], F32, tag="mm")
                        for wsb in (w1h_sb, w1l_sb):
                            nc.tensor.matmul(
                                ph[:], wsb[:, :, hm*128:(hm+1)*128],
                                u8[s][nt][:],
                                start=(wsb is w1h_sb), stop=(wsb is w1l_sb),
                                perf_mode=DROW)
                        if 2*p + i >= 8 - relu_dve:
                            nc.vector.tensor_scalar(
                                h8[:, i, :], ph[:], b1c_sb[:, hm:hm+1], 0.0,
                                OP.add, OP.max)
                        else:
                            nc.scalar.activation(out=h8[:, i, :], in_=ph[:],
                                                 func=ACTF.Relu,
                                                 bias=b1c_sb[:, hm:hm+1])
                    if p < min6_dve:
                        nc.vector.tensor_scalar(h8[:], h8[:], 6.0, None,
                                                OP.min)
                    else:
                        nc.gpsimd.tensor_scalar(h8[:], h8[:], 6.0, None,
                                                OP.min)
                    hps.append(h8)
                return hps

            def mlp_fc2(hps):
                outs = []
                for mh in range(2):
                    p2 = psL.tile([128, 512], F32, tag="lv")
                    for p in range(4):
                        for k, wsb in enumerate((w2h_sb, w2l_sb)):
                            nc.tensor.matmul(
                                p2[:], wsb[:, 2*p:2*p+2, mh*128:(mh+1)*128],
                                hps[p][:], start=(p == 0 and k == 0),
                                stop=(p == 3 and k == 1),
                                perf_mode=DROW)
                    outs.append(p2)
                return outs

            def mlp_y_unit(nt):
                """y2 = mlp(u_y); y3 = y + y2 -> DRAM (re-read in the x loop,
                same sync queue so the round trip is ordered)."""
                hs_y = mlp_fc1(1, nt, min6_dve=2, relu_dve=0)
                y2p = mlp_fc2(hs_y)
                for mh in range(2):
                    yin = tio.tile([128, 512], F32, tag="zin", bufs=6)
                    nc.sync.dma_start(out=yin[:],
                                      in_=d["yb"][mh*128:(mh+1)*128,
                                                  nt*512:(nt+1)*512])
                    y3t = tio.tile([128, 512], F32, tag="y3t", bufs=4)
                    nc.vector.scalar_tensor_tensor(
                        y3t[:], y2p[mh][:], b2c_sb[:, mh:mh+1], yin[:],
                        OP.add, OP.add)
                    nc.sync.dma_start(out=y3o[mh*128:(mh+1)*128,
                                              nt*512:(nt+1)*512], in_=y3t[:])

            st0, st1 = {}, {}
            sepconv_x()
            light_a(0, st0,
                    filler=lambda u: (pw_unit((u // 2) // 4,
                                              4 + (u // 2) % 4, alt_copy=True)
                                      if u % 2 == 0 else None),
                    prefill=lambda: [pw_unit(mh, nt, alt_copy=True)
                                     for mh in range(2) for nt in range(4)])
            light_chain(0, st0)
            sepconv_dw1()              # PE filler for chain(0) latency
            light_chain_post(0, st0)
            def la1_filler(u):
                if u % 4 == 0 and u < 12:
                    light_b(0, st0, [u // 4 + 3])
                elif u == 14:
                    mlp_y_unit(0)
            light_a(1, st1, filler=la1_filler,
                    prefill=lambda: light_b(0, st0, range(0, 3)))
            light_b(0, st0, range(6, NT512))
            light_chain(1, st1)
            for nt in range(1, NT512):  # mlp-Y fills chain(1) latency
                mlp_y_unit(nt)
                light_post_piece(1, nt - 1)
                if nt == 3:
                    light_b(1, st1, [0])
                elif nt == 5:
                    light_b(1, st1, [1])
            light_post_piece(1, NT512 - 1)

            # x-stream mlp + final adds, tile by tile
            for nt in range(NT512):
                if nt + 2 < NT512:
                    light_b(1, st1, [nt + 2])
                ins = []
                for mh in range(2):
                    y3in = tio.tile([128, 512], F32, tag="zin", bufs=6)
                    nc.sync.dma_start(out=y3in[:],
                                      in_=y3o[mh*128:(mh+1)*128,
                                              nt*512:(nt+1)*512])
                    xin = tio.tile([128, 512], F32, tag="zin", bufs=6)
                    nc.sync.dma_start(out=xin[:],
                                      in_=d["xb"][mh*128:(mh+1)*128,
                                                  nt*512:(nt+1)*512])
                    ins.append((y3in, xin))
                hs_x = mlp_fc1(0, nt, min6_dve=4)
                x2p = mlp_fc2(hs_x)
                for mh in range(2):
                    y3in, xin = ins[mh]
                    x3t = tio.tile([128, 512], F32, tag="x3t")
                    nc.vector.scalar_tensor_tensor(
                        x3t[:], x2p[mh][:], b2c_sb[:, mh:mh+1], y3in[:],
                        OP.add, OP.add)
                    nc.gpsimd.tensor_tensor(x3t[:], x3t[:], xin[:], OP.add)
                    nc.sync.dma_start(out=x3o[mh*128:(mh+1)*128,
                                              nt*512:(nt+1)*512], in_=x3t[:])
    nc.compile()
    return nc


def _hilo(a):
    hi = np.clip(a, -240.0, 240.0).astype(F8)
    lo = (a - hi.astype(np.float32)).astype(F8)
    return hi, lo


def _prep_weights(i):
    """Host-side weight folding; returns dict of DMA-ready arrays."""
    bf = lambda a: np.ascontiguousarray(a).astype(BF)
    f8 = lambda a: np.ascontiguousarray(a).astype(F8)
    f32 = lambda a: np.ascontiguousarray(a, dtype=np.float32)

    pw = f32(i["conv_pw_w"])            # [C, C]  out,in
    dw = f32(i["conv_dw_w"])[:, 0]      # [C, 3, 3]
    cs, ct = f32(i["conv_bn_s"]), f32(i["conv_bn_t"])
    ns, nt_ = f32(i["norm_s"]), f32(i["norm_t"])
    dwf = dw * (cs * ns)[:, None, None]                    # fold BN+norm scale
    zbias = ct * ns + nt_                                  # fold BN+norm shift
    dwd = np.zeros((128, 36, 128), np.float32)
    for half in range(2):
        for tap, (dy, dx) in enumerate(TAPS):
            w = dwf[half*128:(half+1)*128, dy+1, dx+1]
            whi, wlo = _hilo(w)
            base = (half*9 + tap) * 2
            dwd[np.arange(128), base, np.arange(128)] = whi.astype(np.float32)
            dwd[np.arange(128), base+1, np.arange(128)] = wlo.astype(np.float32)
    tv = np.concatenate([np.eye(C, dtype=np.float32), f32(i["v_w"]).T], axis=1)
    tv8 = tv.reshape(2, 128, 512).transpose(1, 0, 2)       # [128, kt, 512]
    s1, t1 = f32(i["bn1_s"]), f32(i["bn1_t"])
    s2, t2 = f32(i["bn2_s"]), f32(i["bn2_t"])
    fc1, fb1 = f32(i["fc1_w"]), f32(i["fc1_b"])
    fc2, fb2 = f32(i["fc2_w"]), f32(i["fc2_b"])
    w1f = (s1[:, None] * fc1) * ns[None, :]
    b1f = s1 * (fc1 @ nt_ + fb1) + t1
    w2f = s2[:, None] * fc2
    b2f = s2 * fb2 + t2

    w1T = w1f.T.reshape(2, 128, HID).transpose(1, 0, 2)    # [128, kt, HID]
    w1h, w1l = _hilo(w1T)
    w2T = w2f.T.reshape(8, 128, C).transpose(1, 0, 2)      # [128, k8, C]
    w2h, w2l = _hilo(w2T)
    qT8 = f32(i["q_w"]).T.reshape(2, 128, M).transpose(1, 0, 2)

    return {
        "pwT": bf(pw.T),
        "dwd8": f8(dwd),
        "zbc": f32(zbias.reshape(2, 128).T),
        "tv8": f8(tv8),
        "kT": bf(f32(i["k_w"]).T),
        "qT8": f8(qT8),
        "w1h8": w1h, "w1l8": w1l,
        "b1c": f32(b1f.reshape(8, 128).T),
        "w2h8": w2h, "w2l8": w2l,
        "b2c": f32(b2f.reshape(2, 128).T),
        "i128f": np.eye(128, dtype=np.float32),
        "i128b": bf(np.eye(128)),
        "o128f8": f8(np.ones((128, 1))),
        "ones32": bf(np.ones((1, M))),
    }


def kernel(**inputs):
    x = np.ascontiguousarray(inputs["x"], dtype=np.float32)
    y = np.ascontiguousarray(inputs["y"], dtype=np.float32)
    B = x.shape[0]
    gk = float(np.asarray(inputs["gamma_k"]).reshape(-1)[0])
    gc = float(np.asarray(inputs["gamma_c"]).reshape(-1)[0])

    wmaps = _prep_weights(inputs)
    key = (gk, gc) + tuple(hash(v.tobytes()) for _, v in sorted(wmaps.items()))
    if key not in _CACHE:
        _CACHE.clear()
        _CACHE[key] = _build(gk, gc)
    nc = _CACHE[key]

    in_maps = []
    for b in range(B):
        m = dict(wmaps)
        m["xb"] = x[b].reshape(C, N)
        m["yb"] = y[b].reshape(C, N)
        m["xb16"] = x[b].reshape(C, N).astype(BF)
        m["yb16"] = y[b].reshape(C, N).astype(BF)
        in_maps.append(m)

    res = run_bass_kernel_spmd(nc, in_maps, list(range(B)))
    x3 = np.stack([res.results[b]["x3o"].reshape(C, H, W) for b in range(B)])
    y3 = np.stack([res.results[b]["y3o"].reshape(C, H, W) for b in range(B)])
    return (x3.astype(np.float32), y3.astype(np.float32))


# revision 10
# speedup vs baseline: 1.0556x; 1.0002x over previous
"""Trainium2 Bass kernel for nn_Corssattention (dense_transformer), v2.

Full inputs in, full outputs out. Sharding: data-parallel over batch B=8,
one sample per NeuronCore, weights replicated, no cross-core comm.

v2 over baseline: the three big bf16 matmul groups move to fp8 DoubleRow
with host-side hi/lo weight splitting (whi=f8(w), wlo=f8(w-whi); a DROW
pass per term keeps ~10-bit effective weight precision at 0.5 cyc/row):
  - dw 3x3: 9 DROW passes (tap hi/lo pairs; moving AP = fp8 padded image
    window with a stride-0 subtile broadcast dim) instead of 9 bf16 diags
  - fc1: u in fp8, w1 hi/lo -> 2 DROW passes instead of 2 bf16 passes
  - fc2: w2 hi/lo -> 8 DROW passes (bf16-grade weights, was single fp8)
  - q-proj from z8 via DROW
xin/yin final-residual DMAs ride the SP queue (HWDGE) instead of Pool
SWDGE; drains rebalanced across Act/DVE/Pool.
"""

import numpy as np
import ml_dtypes

import concourse.bass as bass
from concourse import bacc
import concourse.mybir as mybir
import concourse.tile as tile
from concourse.bass_utils import run_bass_kernel_spmd
import bass_rust

F32 = mybir.dt.float32
BF16 = mybir.dt.bfloat16
FP8 = mybir.dt.float8e4
AX = mybir.AxisListType.X
OP = mybir.AluOpType
ACTF = mybir.ActivationFunctionType
DROW = mybir.MatmulPerfMode.DoubleRow

C = 256
H = W = 64
N = H * W            # 4096
M = 32               # KAttention proj dim
HID = 1024
NT512 = N // 512     # 8
NT128 = N // 128     # 32
EPS = 1e-6
PADW = 66            # 64 + 1 pad each side

BF = ml_dtypes.bfloat16
F8 = ml_dtypes.float8_e4m3

TAPS = [(dy, dx) for dy in (-1, 0, 1) for dx in (-1, 0, 1)]

_CACHE = {}


def _build(gk: float, gc: float):
    nc = bacc.Bacc("TRN2", target_bir_lowering=False, debug=False)

    d = {}
    def din(name, shape, dt):
        d[name] = nc.dram_tensor(name, list(shape), dt, kind="ExternalInput").ap()
    din("xb", (C, N), F32)
    din("yb", (C, N), F32)
    din("xb16", (C, N), BF16)
    din("yb16", (C, N), BF16)
    din("pwT", (C, C), BF16)          # conv pw weight, transposed [cin, cout]
    din("dwd8", (128, 36, 128), FP8)  # diag dw [p, ((mh*9+tap)*2+hl), m]
    din("zbc", (128, 2), F32)         # z1 bias as per-partition cols
    din("tv8", (128, 2, 512), FP8)    # [I256 | v_w.T], kt-subtile layout
    din("kT", (C, M), BF16)
    din("qT8", (128, 2, M), FP8)
    din("w1h8", (128, 2, HID), FP8)   # fc1 weights hi, kt-subtile layout
    din("w1l8", (128, 2, HID), FP8)   # fc1 weights lo
    din("b1c", (128, 8), F32)         # fc1 bias cols per hm block
    din("w2h8", (128, 8, C), FP8)     # fc2 weights hi, k-subtile layout
    din("w2l8", (128, 8, C), FP8)     # fc2 weights lo
    din("b2c", (128, 2), F32)         # fc2 bias cols per mh half
    din("i128f", (128, 128), F32)
    din("i128b", (128, 128), BF16)
    din("o128f8", (128, 1), FP8)
    din("ones32", (1, M), BF16)
    x3o = nc.dram_tensor("x3o", [C, N], F32, kind="ExternalOutput").ap()
    y3o = nc.dram_tensor("y3o", [C, N], F32, kind="ExternalOutput").ap()

    with tile.TileContext(nc) as tc:
        with tc.tile_pool(name="wp", bufs=1) as wp, \
             tc.tile_pool(name="tvc", bufs=6) as tvcp, \
             tc.tile_pool(name="tio", bufs=3) as tio, \
             tc.tile_pool(name="tms", bufs=2) as tms, \
             tc.tile_pool(name="hp", bufs=10) as hp, \
             tc.tile_pool(name="psA", bufs=4, space="PSUM") as psA, \
             tc.tile_pool(name="psL", bufs=2, space="PSUM") as psL, \
             tc.tile_pool(name="psR", bufs=2, space="PSUM") as psR:

            # ---- stream-x input load first (startup long pole), chunked.
            zbf = [[wp.tile([128, N // 4], BF16, tag=f"zbf{h}{q}",
                            name=f"zbf{h}{q}") for q in range(4)]
                   for h in range(2)]

            def load_chunk(z_dram, q, dma_eng, cv_eng=None):
                for h in range(2):
                    sl = slice(q * (N // 4), (q + 1) * (N // 4))
                    dma_eng.dma_start(out=zbf[h][q][:],
                                      in_=z_dram[h*128:(h+1)*128, sl])

            def load_input(z_dram, eng=None):
                qengs = [nc.scalar, nc.gpsimd, nc.sync]
                for q in range(4):
                    load_chunk(z_dram, q, qengs[q % len(qengs)])

            def wtile(name, shape, dt, src):
                t = wp.tile(list(shape), dt, tag=name, name=name)
                nc.sync.dma_start(out=t[:], in_=src)
                return t

            pw_sb = [wtile(f"pw{k}", (128, C), BF16, d["pwT"][k*128:(k+1)*128, :])
                     for k in range(2)]

            load_input(d["xb16"])

            # ---- remaining weights ----
            dwd_sb = wtile("dwd8", (128, 36, 128), FP8, d["dwd8"][:, :, :])
            zbc_sb = wtile("zbc", (128, 2), F32, d["zbc"][:, :])
            tv8_sb = wtile("tv8", (128, 2, 512), FP8, d["tv8"][:, :, :])
            kT_sb = [wtile(f"kT{k}", (128, M), BF16, d["kT"][k*128:(k+1)*128, :])
                     for k in range(2)]
            qT8_sb = wtile("qT8", (128, 2, M), FP8, d["qT8"][:, :, :])
            w1h_sb = wtile("w1h8", (128, 2, HID), FP8, d["w1h8"][:, :, :])
            w1l_sb = wtile("w1l8", (128, 2, HID), FP8, d["w1l8"][:, :, :])
            b1c_sb = wtile("b1c", (128, 8), F32, d["b1c"][:, :])
            w2h_sb = wtile("w2h8", (128, 8, C), FP8, d["w2h8"][:, :, :])
            w2l_sb = wtile("w2l8", (128, 8, C), FP8, d["w2l8"][:, :, :])
            b2c_sb = wtile("b2c", (128, 2), F32, d["b2c"][:, :])
            idf_sb = wtile("i128f", (128, 128), F32, d["i128f"][:, :])
            idb_sb = wtile("i128b", (128, 128), BF16, d["i128b"][:, :])
            o128_sb = wtile("o128f8", (128, 1), FP8, d["o128f8"][:, :])
            o32_sb = wtile("ones32", (1, M), BF16, d["ones32"][:, :])

            # ---- persistent tensors ----
            ppad = [wp.tile([128, PADW, PADW], FP8, tag=f"ppad{h}",
                            name=f"ppad{h}") for h in range(2)]
            z1 = {s: [wp.tile([128, N], BF16, tag=f"z1{s}{h}", name=f"z1{s}{h}")
                      for h in range(2)] for s in range(2)}
            z8 = [[wp.tile([128, 2, N // 4], FP8, tag=f"z8{s}c{c}",
                           name=f"z8{s}c{c}") for c in range(4)]
                  for s in range(2)]

            def z8ap(s, lo, size):
                c, off = divmod(lo, N // 4)
                assert off + size <= N // 4
                return z8[s][c][:, :, off:off+size]
            u8 = [[wp.tile([128, 2, 512], FP8, tag=f"u8{s}n{nt}",
                           name=f"u8{s}n{nt}") for nt in range(NT512)]
                  for s in range(2)]
            ax = [wp.tile([128, N], BF16, tag=f"ax{h}", name=f"ax{h}")
                  for h in range(2)]
            KT = [[wp.tile([128, 8, M], BF16, tag=f"KT{s}g{g}",
                           name=f"KT{s}g{g}") for g in range(4)]
                  for s in range(2)]                       # exp intermediate
            KT8 = [[wp.tile([128, 8, M], FP8, tag=f"KT8{s}g{g}",
                            name=f"KT8{s}g{g}") for g in range(4)]
                   for s in range(2)]                      # softplus, fp8
            Q = [[wp.tile([M, N // 4], BF16, tag=f"Q{s}g{g}",
                          name=f"Q{s}g{g}") for g in range(4)]
                 for s in range(2)]

            def Qap(s, lo, size):
                g, off = divmod(lo, N // 4)
                assert off + size <= N // 4
                return Q[s][g][:, off:off+size]
            invFM_t = wp.tile([1, N], BF16, tag="invFM", name="invFM")
            invFM = [invFM_t, invFM_t]

            # zero pad cells of ppad once (interior overwritten each stream)
            for h in range(2):
                p = ppad[h]
                nc.gpsimd.memset(p[:, 0, :], 0.0)
                nc.gpsimd.memset(p[:, PADW - 1, :], 0.0)
                nc.gpsimd.memset(p[:, :, 0], 0.0)
                nc.gpsimd.memset(p[:, :, PADW - 1], 0.0)

            def dw_rhs(mh, nt, dy, dx):
                """4D moving AP over ppad fp8: [128, 2(bcast), 8, 64] window
                at tap (dy,dx) for output row block nt (8 rows)."""
                a = ppad[mh][:]
                off = a.offset + (1 + nt * 8 + dy) * PADW + (1 + dx)
                return bass_rust.AP(a.tensor, off,
                                    [[PADW * PADW, 128], [0, 2], [PADW, 8],
                                     [1, 64]])

            def pw_unit(mh, nt, alt_copy=False):
                ps = psA.tile([128, 512], F32, tag="mm")
                q, r = divmod(nt, 2)
                for kt in range(2):
                    nc.tensor.matmul(
                        ps[:], pw_sb[kt][:, mh*128:(mh+1)*128],
                        zbf[kt][q][:, r*512:(r+1)*512],
                        start=(kt == 0), stop=(kt == 1))
                h0 = nt * 8
                dst = ppad[mh][:, 1+h0:1+h0+8, 1:65]
                src = ps[:].rearrange("p (h w) -> p h w", h=8)
                if alt_copy and mh == 1:
                    nc.vector.tensor_copy(dst, src)
                else:
                    nc.scalar.copy(out=dst, in_=src)

            def dw_unit(s, mh, nt):
                """depthwise 3x3: 9 fp8 DROW passes, weight hi/lo subtile
                pairs, moving = broadcast-subtile window views of ppad."""
                ps = psA.tile([128, 512], F32, tag="mm")
                for t, (dy, dx) in enumerate(TAPS):
                    st = dwd_sb[:, (mh*9 + t)*2:(mh*9 + t)*2 + 2, :]
                    nc.tensor.matmul(
                        ps[:], st, dw_rhs(mh, nt, dy, dx),
                        start=(t == 0), stop=(t == 8), perf_mode=DROW)
                sl = slice(nt*512, (nt+1)*512)
                if s == 0:
                    nc.vector.tensor_scalar(
                        z1[s][mh][:, sl], ps[:], zbc_sb[:, mh:mh+1], None,
                        OP.add)
                else:
                    nc.scalar.activation(
                        out=z1[s][mh][:, sl], in_=ps[:], func=ACTF.Identity,
                        bias=zbc_sb[:, mh:mh+1])
                # Pool cannot read PSUM: derive z8 from z1 (SBUF->SBUF)
                c, off = divmod(nt*512, N // 4)
                nc.gpsimd.tensor_copy(z8[s][c][:, mh, off:off+512],
                                      z1[s][mh][:, sl])

            def q_proj(s, qt):
                pq = psA.tile([M, 512], F32, tag="mm")
                nc.tensor.matmul(pq[:], qT8_sb[:], z8ap(s, qt*512, 512),
                                 start=True, stop=True, perf_mode=DROW)
                nc.scalar.activation(out=Qap(s, qt*512, 512),
                                     in_=pq[:], func=ACTF.Exp)

            def k_group(s, g):
                pk = psR.tile([128, 4, M], F32, tag="rt")
                for j in range(4):
                    nt = 4*g + j
                    for kt in range(2):
                        nc.tensor.matmul(pk[:, j, :],
                                         z1[s][kt][:, nt*128:(nt+1)*128],
                                         kT_sb[kt][:],
                                         start=(kt == 0), stop=(kt == 1))
                nc.scalar.activation(
                    out=KT[s][4*g // 8][:, (4*g) % 8:(4*g) % 8 + 4, :],
                    in_=pk[:], func=ACTF.Exp)

            def sepconv_x():
                for nt in range(NT512):
                    for mh in range(2):
                        pw_unit(mh, nt)
                    if nt % 2 == 1:
                        q = nt // 2
                        load_chunk(d["yb16"], q, nc.sync)
                    if nt >= 1:
                        for mh in range(2):
                            dw_unit(0, mh, nt - 1)
                    if nt >= 2:
                        q_proj(0, nt - 2)
                        k_group(0, nt - 2)
                for mh in range(2):
                    dw_unit(0, mh, NT512 - 1)
                for qt in range(NT512 - 2, NT512):
                    q_proj(0, qt)
                    k_group(0, qt)

            def sepconv_dw1():
                for nt in range(NT512):
                    for mh in range(2):
                        dw_unit(1, mh, nt)
                    if nt >= 1:
                        q_proj(1, nt - 1)
                        k_group(1, nt - 1)
                q_proj(1, NT512 - 1)
                k_group(1, NT512 - 1)

            def light_a(s, st, filler=None, prefill=None):
                if prefill is not None:
                    prefill()
                with tc.tile_critical():
                    for g in range(4):
                        nc.scalar.activation(
                            out=KT8[s][g][:], in_=KT[s][g][:],
                            func=ACTF.Ln, bias=1.0)
                    for g in range(4):
                        nc.scalar.activation(
                            out=Q[s][g][:], in_=Q[s][g][:],
                            func=ACTF.Ln, bias=1.0)
                pe = [psL.tile([128, C], F32, tag="lv", name=f"pe{i}")
                      for i in range(2)]
                pkv = psR.tile([M, C], F32, tag="rt")
                for pr in range(16):
                    if filler is not None:
                        filler(pr)
                    chp = tvcp.tile([128, 2, 512], FP8, tag="tvc")
                    for j in range(2):
                        nt = 2*pr + j
                        ps = psA.tile([128, 512], F32, tag="mm")
                        nc.tensor.matmul(ps[:], z8ap(s, nt*128, 128),
                                         tv8_sb[:], start=True, stop=True,
                                         perf_mode=DROW)
                        if j == 0:
                            nc.vector.tensor_copy(chp[:, j, :], ps[:])
                        else:
                            nc.scalar.copy(out=chp[:, j, :], in_=ps[:])
                    for mh in range(2):
                        nc.tensor.matmul(pe[mh][:],
                                         chp[:, :, mh*128:(mh+1)*128],
                                         chp[:, :, 0:256],
                                         start=(pr == 0), stop=(pr == 15),
                                         perf_mode=DROW)
                    nc.tensor.matmul(
                        pkv[:],
                        KT8[s][(2*pr) // 8][:, (2*pr) % 8:(2*pr) % 8 + 2, :],
                        chp[:, :, 256:512],
                        start=(pr == 0), stop=(pr == 15),
                        perf_mode=DROW)
                st["pe"], st["pkv"] = pe, pkv

            def light_chain(s, st):
                pe, pkv = st["pe"], st["pkv"]
                kv = tms.tile([M, C], BF16, tag="kv", bufs=2)
                nc.vector.tensor_scalar(kv[:], pkv[:], gk, None, OP.mult)
                st["kv"] = kv
                pks = psR.tile([M, 1], F32, tag="rt")
                for nt in range(NT128):
                    nc.tensor.matmul(pks[:],
                                     KT8[s][nt // 8][:, nt % 8, :],
                                     o128_sb[:],
                                     start=(nt == 0), stop=(nt == NT128 - 1))
                ksum = tms.tile([M, 1], BF16, tag="ksum")
                nc.vector.tensor_scalar(ksum[:], pks[:], EPS, None, OP.add)
                pdT = psR.tile([128, NT128], F32, tag="rt")
                for t in range(NT128):
                    nc.tensor.matmul(pdT[:, t:t+1], Qap(s, t*128, 128),
                                     ksum[:], start=True, stop=True)
                invA = tms.tile([128, NT128], F32, tag="invA")
                nc.vector.reciprocal(invA[:], pdT[:])
                piT = psR.tile([M, 128], F32, tag="rt")
                nc.tensor.transpose(piT[:], invA[:], idf_sb[:])
                invT = tms.tile([NT128, 128], BF16, tag="invT")
                nc.vector.tensor_copy(invT[:], piT[:])
                nc.sync.dma_start(out=invFM[s][0:1, :], in_=invT[:])
                att = [tms.tile([128, C], BF16, tag=f"att{mh}", name=f"att{mh}")
                       for mh in range(2)]
                for mh in range(2):
                    rmax = tms.tile([128, 1], F32, tag="rmax")
                    nc.vector.tensor_reduce(rmax[:], pe[mh][:], axis=AX,
                                            op=OP.min)
                    tdiff = tms.tile([128, C], F32, tag="tdiff")
                    nc.vector.tensor_scalar(tdiff[:], pe[mh][:], rmax[:], 30.0,
                                            OP.subtract, OP.min)
                    ex = tms.tile([128, C], F32, tag="ex")
                    nc.scalar.activation(out=ex[:], in_=tdiff[:], func=ACTF.Exp,
                                         scale=-1.0)
                    rsum = tms.tile([128, 1], F32, tag="rsum")
                    nc.vector.reduce_sum(rsum[:], ex[:], axis=AX)
                    rinv = tms.tile([128, 1], F32, tag="rinv")
                    nc.vector.reciprocal(rinv[:], rsum[:])
                    nc.vector.tensor_scalar(att[mh][:], ex[:], rinv[:], gc,
                                            OP.mult, OP.mult)
                att8T = tms.tile([128, 2, C], FP8, tag="att8T", bufs=2)
                for mh in range(2):
                    for kt in range(2):
                        pat = psR.tile([128, 128], BF16, tag="rt")
                        nc.tensor.transpose(pat[:],
                                            att[mh][:, kt*128:(kt+1)*128],
                                            idb_sb[:])
                        nc.vector.tensor_copy(
                            att8T[:, kt, mh*128:(mh+1)*128], pat[:])
                st["att8T"] = att8T

            def light_post_piece(s, nt):
                pib = psA.tile([M, 512], F32, tag="mm")
                nc.tensor.matmul(pib[:], o32_sb[0:1, :],
                                 invFM[s][0:1, nt*512:(nt+1)*512],
                                 start=True, stop=True)
                nc.vector.tensor_tensor(Qap(s, nt*512, 512),
                                        Qap(s, nt*512, 512), pib[:],
                                        OP.mult)

            def light_chain_post(s, st):
                for nt in range(NT512):
                    light_post_piece(s, nt)

            def light_b(s, st, nts):
                """wv + catt in one PSUM; combine a = 2*z1 + psum.
                s=0: store into ax.  s=1: ay consumed on the fly ->
                u8[0] = fp8(x1*ay), u8[1] = fp8(y1*ax)."""
                z = z1[s]
                kv, att8T = st["kv"], st["att8T"]
                for nt in nts:
                    pos = []
                    for mh in range(2):
                        po = psA.tile([128, 512], F32, tag="mm")
                        nc.tensor.matmul(po[:], kv[:, mh*128:(mh+1)*128],
                                         Qap(s, nt*512, 512),
                                         start=True, stop=False)
                        nc.tensor.matmul(po[:],
                                         att8T[:, :, mh*128:(mh+1)*128],
                                         z8ap(s, nt*512, 512),
                                         start=False, stop=True,
                                         perf_mode=DROW)
                        pos.append(po)
                    for mh in range(2):
                        sl = slice(nt*512, (nt+1)*512)
                        if s == 0:
                            nc.vector.scalar_tensor_tensor(
                                ax[mh][:, sl], z[mh][:, sl], 2.0, pos[mh][:],
                                OP.mult, OP.add)
                            # u_y = y1*ax ready as soon as ax lands (Pool,
                            # idle in this phase) -> mlp-Y can run early
                            nc.gpsimd.tensor_tensor(u8[1][nt][:, mh, :],
                                                    z1[1][mh][:, sl],
                                                    ax[mh][:, sl], OP.mult)
                        else:
                            ay_t = tvcp.tile([128, 512], BF16, tag="ayt")
                            nc.vector.scalar_tensor_tensor(
                                ay_t[:], z[mh][:, sl], 2.0, pos[mh][:],
                                OP.mult, OP.add)
                            nc.gpsimd.tensor_tensor(u8[0][nt][:, mh, :],
                                                    z1[0][mh][:, sl], ay_t[:],
                                                    OP.mult)

            # mlp pieces -----------------------------------------------------
            def mlp_fc1(s, nt, min6_dve=3, relu_dve=0):
                """hidden = relu6(fc1(u8) + b): 2 DROW passes per 128-block
                (w1 hi + lo); relu+bias drains on Act (last `relu_dve`
                halves on DVE), min6 split DVE/Pool."""
                hps = []
                for p in range(4):
                    h8 = hp.tile([128, 2, 512], FP8, tag="h")
                    for i in range(2):
                        hm = 2*p + i
                        ph = psA.tile([128, 512], F32, tag="mm")
                        for wsb in (w1h_sb, w1l_sb):
                            nc.tensor.matmul(
                                ph[:], wsb[:, :, hm*128:(hm+1)*128],
                                u8[s][nt][:],
                                start=(wsb is w1h_sb), stop=(wsb is w1l_sb),
                                perf_mode=DROW)
                        if 2*p + i >= 8 - relu_dve:
                            nc.vector.tensor_scalar(
                                h8[:, i, :], ph[:], b1c_sb[:, hm:hm+1], 0.0,
                                OP.add, OP.max)
                        else:
                            nc.scalar.activation(out=h8[:, i, :], in_=ph[:],
                                                 func=ACTF.Relu,
                                                 bias=b1c_sb[:, hm:hm+1])
                    if p < min6_dve:
                        nc.vector.tensor_scalar(h8[:], h8[:], 6.0, None,
                                                OP.min)
                    else:
                        nc.gpsimd.tensor_scalar(h8[:], h8[:], 6.0, None,
                                                OP.min)
                    hps.append(h8)
                return hps

            def mlp_fc2(hps):
                outs = []
                for mh in range(2):
                    p2 = psL.tile([128, 512], F32, tag="lv")
                    for p in range(4):
                        for k, wsb in enumerate((w2h_sb, w2l_sb)):
                            nc.tensor.matmul(
                                p2[:], wsb[:, 2*p:2*p+2, mh*128:(mh+1)*128],
                                hps[p][:], start=(p == 0 and k == 0),
                                stop=(p == 3 and k == 1),
                                perf_mode=DROW)
                    outs.append(p2)
                return outs

            def mlp_y_unit(nt):
                """y2 = mlp(u_y); y3 = y + y2 -> DRAM (re-read in the x loop,
                same sync queue so the round trip is ordered)."""
                hs_y = mlp_fc1(1, nt, min6_dve=2, relu_dve=0)
                y2p = mlp_fc2(hs_y)
                for mh in range(2):
                    yin = tio.tile([128, 512], F32, tag="zin", bufs=6)
                    nc.sync.dma_start(out=yin[:],
                                      in_=d["yb"][mh*128:(mh+1)*128,
                                                  nt*512:(nt+1)*512])
                    y3t = tio.tile([128, 512], F32, tag="y3t", bufs=4)
                    nc.vector.scalar_tensor_tensor(
                        y3t[:], y2p[mh][:], b2c_sb[:, mh:mh+1], yin[:],
                        OP.add, OP.add)
                    nc.sync.dma_start(out=y3o[mh*128:(mh+1)*128,
                                              nt*512:(nt+1)*512], in_=y3t[:])

            st0, st1 = {}, {}
            sepconv_x()
            light_a(0, st0,
                    filler=lambda u: (pw_unit((u // 2) // 4,
                                              4 + (u // 2) % 4, alt_copy=True)
                                      if u % 2 == 0 else None),
                    prefill=lambda: [pw_unit(mh, nt, alt_copy=True)
                                     for mh in range(2) for nt in range(4)])
            light_chain(0, st0)
            sepconv_dw1()              # PE filler for chain(0) latency
            light_chain_post(0, st0)
            def la1_filler(u):
                if u % 4 == 0 and u < 12:
                    light_b(0, st0, [u // 4 + 3])
                elif u == 12:
                    mlp_y_unit(0)
            light_a(1, st1, filler=la1_filler,
                    prefill=lambda: light_b(0, st0, range(0, 3)))
            light_b(0, st0, range(6, NT512))
            light_chain(1, st1)
            for nt in range(1, NT512):  # mlp-Y fills chain(1) latency
                mlp_y_unit(nt)
                light_post_piece(1, nt - 1)
                if nt == 3:
                    light_b(1, st1, [0])
                elif nt == 5:
                    light_b(1, st1, [1])
            light_post_piece(1, NT512 - 1)

            # x-stream mlp + final adds, tile by tile
            for nt in range(NT512):
                if nt + 2 < NT512:
                    light_b(1, st1, [nt + 2])
                ins = []
                for mh in range(2):
                    y3in = tio.tile([128, 512], F32, tag="zin", bufs=6)
                    nc.sync.dma_start(out=y3in[:],
                                      in_=y3o[mh*128:(mh+1)*128,
                                              nt*512:(nt+1)*512])
                    xin = tio.tile([128, 512], F32, tag="zin", bufs=6)
                    nc.sync.dma_start(out=xin[:],
                                      in_=d["xb"][mh*128:(mh+1)*128,
                                                  nt*512:(nt+1)*512])
                    ins.append((y3in, xin))
                hs_x = mlp_fc1(0, nt, min6_dve=4)
                x2p = mlp_fc2(hs_x)
                for mh in range(2):
                    y3in, xin = ins[mh]
                    x3t = tio.tile([128, 512], F32, tag="x3t")
                    nc.vector.scalar_tensor_tensor(
                        x3t[:], x2p[mh][:], b2c_sb[:, mh:mh+1], y3in[:],
                        OP.add, OP.add)
                    nc.gpsimd.tensor_tensor(x3t[:], x3t[:], xin[:], OP.add)
                    nc.sync.dma_start(out=x3o[mh*128:(mh+1)*128,
                                              nt*512:(nt+1)*512], in_=x3t[:])
    nc.compile()
    return nc


def _hilo(a):
    hi = np.clip(a, -240.0, 240.0).astype(F8)
    lo = (a - hi.astype(np.float32)).astype(F8)
    return hi, lo


def _prep_weights(i):
    """Host-side weight folding; returns dict of DMA-ready arrays."""
    bf = lambda a: np.ascontiguousarray(a).astype(BF)
    f8 = lambda a: np.ascontiguousarray(a).astype(F8)
    f32 = lambda a: np.ascontiguousarray(a, dtype=np.float32)

    pw = f32(i["conv_pw_w"])            # [C, C]  out,in
    dw = f32(i["conv_dw_w"])[:, 0]      # [C, 3, 3]
    cs, ct = f32(i["conv_bn_s"]), f32(i["conv_bn_t"])
    ns, nt_ = f32(i["norm_s"]), f32(i["norm_t"])
    dwf = dw * (cs * ns)[:, None, None]                    # fold BN+norm scale
    zbias = ct * ns + nt_                                  # fold BN+norm shift
    dwd = np.zeros((128, 36, 128), np.float32)
    for half in range(2):
        for tap, (dy, dx) in enumerate(TAPS):
            w = dwf[half*128:(half+1)*128, dy+1, dx+1]
            whi, wlo = _hilo(w)
            base = (half*9 + tap) * 2
            dwd[np.arange(128), base, np.arange(128)] = whi.astype(np.float32)
            dwd[np.arange(128), base+1, np.arange(128)] = wlo.astype(np.float32)
    tv = np.concatenate([np.eye(C, dtype=np.float32), f32(i["v_w"]).T], axis=1)
    tv8 = tv.reshape(2, 128, 512).transpose(1, 0, 2)       # [128, kt, 512]
    s1, t1 = f32(i["bn1_s"]), f32(i["bn1_t"])
    s2, t2 = f32(i["bn2_s"]), f32(i["bn2_t"])
    fc1, fb1 = f32(i["fc1_w"]), f32(i["fc1_b"])
    fc2, fb2 = f32(i["fc2_w"]), f32(i["fc2_b"])
    w1f = (s1[:, None] * fc1) * ns[None, :]
    b1f = s1 * (fc1 @ nt_ + fb1) + t1
    w2f = s2[:, None] * fc2
    b2f = s2 * fb2 + t2

    w1T = w1f.T.reshape(2, 128, HID).transpose(1, 0, 2)    # [128, kt, HID]
    w1h, w1l = _hilo(w1T)
    w2T = w2f.T.reshape(8, 128, C).transpose(1, 0, 2)      # [128, k8, C]
    w2h, w2l = _hilo(w2T)
    qT8 = f32(i["q_w"]).T.reshape(2, 128, M).transpose(1, 0, 2)

    return {
        "pwT": bf(pw.T),
        "dwd8": f8(dwd),
        "zbc": f32(zbias.reshape(2, 128).T),
        "tv8": f8(tv8),
        "kT": bf(f32(i["k_w"]).T),
        "qT8": f8(qT8),
        "w1h8": w1h, "w1l8": w1l,
        "b1c": f32(b1f.reshape(8, 128).T),
        "w2h8": w2h, "w2l8": w2l,
        "b2c": f32(b2f.reshape(2, 128).T),
        "i128f": np.eye(128, dtype=np.float32),
        "i128b": bf(np.eye(128)),
        "o128f8": f8(np.ones((128, 1))),
        "ones32": bf(np.ones((1, M))),
    }


def kernel(**inputs):
    x = np.ascontiguousarray(inputs["x"], dtype=np.float32)
    y = np.ascontiguousarray(inputs["y"], dtype=np.float32)
    B = x.shape[0]
    gk = float(np.asarray(inputs["gamma_k"]).reshape(-1)[0])
    gc = float(np.asarray(inputs["gamma_c"]).reshape(-1)[0])

    wmaps = _prep_weights(inputs)
    key = (gk, gc) + tuple(hash(v.tobytes()) for _, v in sorted(wmaps.items()))
    if key not in _CACHE:
        _CACHE.clear()
        _CACHE[key] = _build(gk, gc)
    nc = _CACHE[key]

    in_maps = []
    for b in range(B):
        m = dict(wmaps)
        m["xb"] = x[b].reshape(C, N)
        m["yb"] = y[b].reshape(C, N)
        m["xb16"] = x[b].reshape(C, N).astype(BF)
        m["yb16"] = y[b].reshape(C, N).astype(BF)
        in_maps.append(m)

    res = run_bass_kernel_spmd(nc, in_maps, list(range(B)))
    x3 = np.stack([res.results[b]["x3o"].reshape(C, H, W) for b in range(B)])
    y3 = np.stack([res.results[b]["y3o"].reshape(C, H, W) for b in range(B)])
    return (x3.astype(np.float32), y3.astype(np.float32))


# revision 11
# speedup vs baseline: 1.0595x; 1.0037x over previous
"""Trainium2 Bass kernel for nn_Corssattention (dense_transformer), v2.

Full inputs in, full outputs out. Sharding: data-parallel over batch B=8,
one sample per NeuronCore, weights replicated, no cross-core comm.

v2 over baseline: the three big bf16 matmul groups move to fp8 DoubleRow
with host-side hi/lo weight splitting (whi=f8(w), wlo=f8(w-whi); a DROW
pass per term keeps ~10-bit effective weight precision at 0.5 cyc/row):
  - dw 3x3: 9 DROW passes (tap hi/lo pairs; moving AP = fp8 padded image
    window with a stride-0 subtile broadcast dim) instead of 9 bf16 diags
  - fc1: u in fp8, w1 hi/lo -> 2 DROW passes instead of 2 bf16 passes
  - fc2: w2 hi/lo -> 8 DROW passes (bf16-grade weights, was single fp8)
  - q-proj from z8 via DROW
xin/yin final-residual DMAs ride the SP queue (HWDGE) instead of Pool
SWDGE; drains rebalanced across Act/DVE/Pool.
"""

import numpy as np
import ml_dtypes

import concourse.bass as bass
from concourse import bacc
import concourse.mybir as mybir
import concourse.tile as tile
from concourse.bass_utils import run_bass_kernel_spmd
import bass_rust

F32 = mybir.dt.float32
BF16 = mybir.dt.bfloat16
FP8 = mybir.dt.float8e4
AX = mybir.AxisListType.X
OP = mybir.AluOpType
ACTF = mybir.ActivationFunctionType
DROW = mybir.MatmulPerfMode.DoubleRow

C = 256
H = W = 64
N = H * W            # 4096
M = 32               # KAttention proj dim
HID = 1024
NT512 = N // 512     # 8
NT128 = N // 128     # 32
EPS = 1e-6
PADW = 66            # 64 + 1 pad each side

BF = ml_dtypes.bfloat16
F8 = ml_dtypes.float8_e4m3

TAPS = [(dy, dx) for dy in (-1, 0, 1) for dx in (-1, 0, 1)]

_CACHE = {}


def _build(gk: float, gc: float):
    nc = bacc.Bacc("TRN2", target_bir_lowering=False, debug=False)

    d = {}
    def din(name, shape, dt):
        d[name] = nc.dram_tensor(name, list(shape), dt, kind="ExternalInput").ap()
    din("xb", (C, N), F32)
    din("yb", (C, N), F32)
    din("xb16", (C, N), BF16)
    din("yb16", (C, N), BF16)
    din("pwT", (C, C), BF16)          # conv pw weight, transposed [cin, cout]
    din("dwd8", (128, 36, 128), FP8)  # diag dw [p, ((mh*9+tap)*2+hl), m]
    din("zbc", (128, 2), F32)         # z1 bias as per-partition cols
    din("tv8", (128, 2, 512), FP8)    # [I256 | v_w.T], kt-subtile layout
    din("kT", (C, M), BF16)
    din("qT8", (128, 2, M), FP8)
    din("kT8w", (128, 2, M), FP8)
    din("w1h8", (128, 2, HID), FP8)   # fc1 weights hi, kt-subtile layout
    din("w1l8", (128, 2, HID), FP8)   # fc1 weights lo
    din("b1c", (128, 8), F32)         # fc1 bias cols per hm block
    din("w2h8", (128, 8, C), FP8)     # fc2 weights hi, k-subtile layout
    din("w2l8", (128, 8, C), FP8)     # fc2 weights lo
    din("b2c", (128, 2), F32)         # fc2 bias cols per mh half
    din("i128f", (128, 128), F32)
    din("i128b", (128, 128), BF16)
    din("o128f8", (128, 1), FP8)
    din("ones32", (1, M), BF16)
    x3o = nc.dram_tensor("x3o", [C, N], F32, kind="ExternalOutput").ap()
    y3o = nc.dram_tensor("y3o", [C, N], F32, kind="ExternalOutput").ap()

    with tile.TileContext(nc) as tc:
        with tc.tile_pool(name="wp", bufs=1) as wp, \
             tc.tile_pool(name="tvc", bufs=6) as tvcp, \
             tc.tile_pool(name="tio", bufs=3) as tio, \
             tc.tile_pool(name="tms", bufs=2) as tms, \
             tc.tile_pool(name="hp", bufs=10) as hp, \
             tc.tile_pool(name="psA", bufs=4, space="PSUM") as psA, \
             tc.tile_pool(name="psL", bufs=2, space="PSUM") as psL, \
             tc.tile_pool(name="psR", bufs=2, space="PSUM") as psR:

            # ---- stream-x input load first (startup long pole), chunked.
            zbf = [[wp.tile([128, N // 4], BF16, tag=f"zbf{h}{q}",
                            name=f"zbf{h}{q}") for q in range(4)]
                   for h in range(2)]

            def load_chunk(z_dram, q, dma_eng, cv_eng=None):
                for h in range(2):
                    sl = slice(q * (N // 4), (q + 1) * (N // 4))
                    dma_eng.dma_start(out=zbf[h][q][:],
                                      in_=z_dram[h*128:(h+1)*128, sl])

            def load_input(z_dram, eng=None):
                qengs = [nc.scalar, nc.gpsimd, nc.sync]
                for q in range(4):
                    load_chunk(z_dram, q, qengs[q % len(qengs)])

            def wtile(name, shape, dt, src):
                t = wp.tile(list(shape), dt, tag=name, name=name)
                nc.sync.dma_start(out=t[:], in_=src)
                return t

            pw_sb = [wtile(f"pw{k}", (128, C), BF16, d["pwT"][k*128:(k+1)*128, :])
                     for k in range(2)]

            load_input(d["xb16"])

            # ---- remaining weights ----
            dwd_sb = wtile("dwd8", (128, 36, 128), FP8, d["dwd8"][:, :, :])
            zbc_sb = wtile("zbc", (128, 2), F32, d["zbc"][:, :])
            tv8_sb = wtile("tv8", (128, 2, 512), FP8, d["tv8"][:, :, :])
            kT_sb = [wtile(f"kT{k}", (128, M), BF16, d["kT"][k*128:(k+1)*128, :])
                     for k in range(2)]
            qT8_sb = wtile("qT8", (128, 2, M), FP8, d["qT8"][:, :, :])
            kT8w_sb = wtile("kT8w", (128, 2, M), FP8, d["kT8w"][:, :, :])
            w1h_sb = wtile("w1h8", (128, 2, HID), FP8, d["w1h8"][:, :, :])
            w1l_sb = wtile("w1l8", (128, 2, HID), FP8, d["w1l8"][:, :, :])
            b1c_sb = wtile("b1c", (128, 8), F32, d["b1c"][:, :])
            w2h_sb = wtile("w2h8", (128, 8, C), FP8, d["w2h8"][:, :, :])
            w2l_sb = wtile("w2l8", (128, 8, C), FP8, d["w2l8"][:, :, :])
            b2c_sb = wtile("b2c", (128, 2), F32, d["b2c"][:, :])
            idf_sb = wtile("i128f", (128, 128), F32, d["i128f"][:, :])
            idb_sb = wtile("i128b", (128, 128), BF16, d["i128b"][:, :])
            o128_sb = wtile("o128f8", (128, 1), FP8, d["o128f8"][:, :])
            o32_sb = wtile("ones32", (1, M), BF16, d["ones32"][:, :])

            # ---- persistent tensors ----
            ppad = [wp.tile([128, PADW, PADW], FP8, tag=f"ppad{h}",
                            name=f"ppad{h}") for h in range(2)]
            z1 = {s: [wp.tile([128, N], BF16, tag=f"z1{s}{h}", name=f"z1{s}{h}")
                      for h in range(2)] for s in range(2)}
            z8 = [[wp.tile([128, 2, N // 4], FP8, tag=f"z8{s}c{c}",
                           name=f"z8{s}c{c}") for c in range(4)]
                  for s in range(2)]

            def z8ap(s, lo, size):
                c, off = divmod(lo, N // 4)
                assert off + size <= N // 4
                return z8[s][c][:, :, off:off+size]
            u8 = [[wp.tile([128, 2, 512], FP8, tag=f"u8{s}n{nt}",
                           name=f"u8{s}n{nt}") for nt in range(NT512)]
                  for s in range(2)]
            ax = [wp.tile([128, N], BF16, tag=f"ax{h}", name=f"ax{h}")
                  for h in range(2)]
            KT = [[wp.tile([128, 8, M], BF16, tag=f"KT{s}g{g}",
                           name=f"KT{s}g{g}") for g in range(4)]
                  for s in range(2)]                       # exp intermediate
            KT8 = [[wp.tile([128, 8, M], FP8, tag=f"KT8{s}g{g}",
                            name=f"KT8{s}g{g}") for g in range(4)]
                   for s in range(2)]                      # softplus, fp8
            Q = [[wp.tile([M, N // 4], BF16, tag=f"Q{s}g{g}",
                          name=f"Q{s}g{g}") for g in range(4)]
                 for s in range(2)]

            def Qap(s, lo, size):
                g, off = divmod(lo, N // 4)
                assert off + size <= N // 4
                return Q[s][g][:, off:off+size]
            invFM_t = wp.tile([1, N], BF16, tag="invFM", name="invFM")
            invFM = [invFM_t, invFM_t]

            # zero pad cells of ppad once (interior overwritten each stream)
            for h in range(2):
                p = ppad[h]
                nc.gpsimd.memset(p[:, 0, :], 0.0)
                nc.gpsimd.memset(p[:, PADW - 1, :], 0.0)
                nc.gpsimd.memset(p[:, :, 0], 0.0)
                nc.gpsimd.memset(p[:, :, PADW - 1], 0.0)

            def dw_rhs(mh, nt, dy, dx):
                """4D moving AP over ppad fp8: [128, 2(bcast), 8, 64] window
                at tap (dy,dx) for output row block nt (8 rows)."""
                a = ppad[mh][:]
                off = a.offset + (1 + nt * 8 + dy) * PADW + (1 + dx)
                return bass_rust.AP(a.tensor, off,
                                    [[PADW * PADW, 128], [0, 2], [PADW, 8],
                                     [1, 64]])

            def pw_unit(mh, nt, alt_copy=False):
                ps = psA.tile([128, 512], F32, tag="mm")
                q, r = divmod(nt, 2)
                for kt in range(2):
                    nc.tensor.matmul(
                        ps[:], pw_sb[kt][:, mh*128:(mh+1)*128],
                        zbf[kt][q][:, r*512:(r+1)*512],
                        start=(kt == 0), stop=(kt == 1))
                h0 = nt * 8
                dst = ppad[mh][:, 1+h0:1+h0+8, 1:65]
                src = ps[:].rearrange("p (h w) -> p h w", h=8)
                if alt_copy and mh == 1:
                    nc.vector.tensor_copy(dst, src)
                else:
                    nc.scalar.copy(out=dst, in_=src)

            def dw_unit(s, mh, nt):
                """depthwise 3x3: 9 fp8 DROW passes, weight hi/lo subtile
                pairs, moving = broadcast-subtile window views of ppad."""
                ps = psA.tile([128, 512], F32, tag="mm")
                for t, (dy, dx) in enumerate(TAPS):
                    st = dwd_sb[:, (mh*9 + t)*2:(mh*9 + t)*2 + 2, :]
                    nc.tensor.matmul(
                        ps[:], st, dw_rhs(mh, nt, dy, dx),
                        start=(t == 0), stop=(t == 8), perf_mode=DROW)
                sl = slice(nt*512, (nt+1)*512)
                if s == 0:
                    nc.vector.tensor_scalar(
                        z1[s][mh][:, sl], ps[:], zbc_sb[:, mh:mh+1], None,
                        OP.add)
                else:
                    nc.scalar.activation(
                        out=z1[s][mh][:, sl], in_=ps[:], func=ACTF.Identity,
                        bias=zbc_sb[:, mh:mh+1])
                # Pool cannot read PSUM: derive z8 from z1 (SBUF->SBUF)
                c, off = divmod(nt*512, N // 4)
                nc.gpsimd.tensor_copy(z8[s][c][:, mh, off:off+512],
                                      z1[s][mh][:, sl])

            def q_proj(s, qt):
                pq = psA.tile([M, 512], F32, tag="mm")
                nc.tensor.matmul(pq[:], qT8_sb[:], z8ap(s, qt*512, 512),
                                 start=True, stop=True, perf_mode=DROW)
                nc.scalar.activation(out=Qap(s, qt*512, 512),
                                     in_=pq[:], func=ACTF.Exp)

            def k_group(s, g):
                pk = psR.tile([128, 4, M], F32, tag="rt")
                for j in range(4):
                    nt = 4*g + j
                    nc.tensor.matmul(pk[:, j, :], z8ap(s, nt*128, 128),
                                     kT8w_sb[:], start=True, stop=True,
                                     perf_mode=DROW)
                nc.scalar.activation(
                    out=KT[s][4*g // 8][:, (4*g) % 8:(4*g) % 8 + 4, :],
                    in_=pk[:], func=ACTF.Exp)

            def sepconv_x():
                for nt in range(NT512):
                    for mh in range(2):
                        pw_unit(mh, nt)
                    if nt % 2 == 1:
                        q = nt // 2
                        load_chunk(d["yb16"], q, nc.sync)
                    if nt >= 1:
                        for mh in range(2):
                            dw_unit(0, mh, nt - 1)
                    if nt >= 2:
                        q_proj(0, nt - 2)
                        k_group(0, nt - 2)
                for mh in range(2):
                    dw_unit(0, mh, NT512 - 1)
                for qt in range(NT512 - 2, NT512):
                    q_proj(0, qt)
                    k_group(0, qt)

            def sepconv_dw1():
                for nt in range(NT512):
                    for mh in range(2):
                        dw_unit(1, mh, nt)
                    if nt >= 1:
                        q_proj(1, nt - 1)
                        k_group(1, nt - 1)
                q_proj(1, NT512 - 1)
                k_group(1, NT512 - 1)

            def light_a(s, st, filler=None, prefill=None):
                if prefill is not None:
                    prefill()
                with tc.tile_critical():
                    for g in range(4):
                        nc.scalar.activation(
                            out=KT8[s][g][:], in_=KT[s][g][:],
                            func=ACTF.Ln, bias=1.0)
                    for g in range(4):
                        nc.scalar.activation(
                            out=Q[s][g][:], in_=Q[s][g][:],
                            func=ACTF.Ln, bias=1.0)
                pe = [psL.tile([128, C], F32, tag="lv", name=f"pe{i}")
                      for i in range(2)]
                pkv = psR.tile([M, C], F32, tag="rt")
                for pr in range(16):
                    if filler is not None:
                        filler(pr)
                    chp = tvcp.tile([128, 2, 512], FP8, tag="tvc")
                    for j in range(2):
                        nt = 2*pr + j
                        ps = psA.tile([128, 512], F32, tag="mm")
                        nc.tensor.matmul(ps[:], z8ap(s, nt*128, 128),
                                         tv8_sb[:], start=True, stop=True,
                                         perf_mode=DROW)
                        if j == 0:
                            nc.vector.tensor_copy(chp[:, j, :], ps[:])
                        else:
                            nc.scalar.copy(out=chp[:, j, :], in_=ps[:])
                    for mh in range(2):
                        nc.tensor.matmul(pe[mh][:],
                                         chp[:, :, mh*128:(mh+1)*128],
                                         chp[:, :, 0:256],
                                         start=(pr == 0), stop=(pr == 15),
                                         perf_mode=DROW)
                    nc.tensor.matmul(
                        pkv[:],
                        KT8[s][(2*pr) // 8][:, (2*pr) % 8:(2*pr) % 8 + 2, :],
                        chp[:, :, 256:512],
                        start=(pr == 0), stop=(pr == 15),
                        perf_mode=DROW)
                st["pe"], st["pkv"] = pe, pkv

            def light_chain(s, st):
                pe, pkv = st["pe"], st["pkv"]
                kv = tms.tile([M, C], BF16, tag="kv", bufs=2)
                nc.vector.tensor_scalar(kv[:], pkv[:], gk, None, OP.mult)
                st["kv"] = kv
                pks = psR.tile([M, 1], F32, tag="rt")
                for nt in range(NT128):
                    nc.tensor.matmul(pks[:],
                                     KT8[s][nt // 8][:, nt % 8, :],
                                     o128_sb[:],
                                     start=(nt == 0), stop=(nt == NT128 - 1))
                ksum = tms.tile([M, 1], BF16, tag="ksum")
                nc.vector.tensor_scalar(ksum[:], pks[:], EPS, None, OP.add)
                pdT = psR.tile([128, NT128], F32, tag="rt")
                for t in range(NT128):
                    nc.tensor.matmul(pdT[:, t:t+1], Qap(s, t*128, 128),
                                     ksum[:], start=True, stop=True)
                invA = tms.tile([128, NT128], F32, tag="invA")
                nc.vector.reciprocal(invA[:], pdT[:])
                piT = psR.tile([M, 128], F32, tag="rt")
                nc.tensor.transpose(piT[:], invA[:], idf_sb[:])
                invT = tms.tile([NT128, 128], BF16, tag="invT")
                nc.vector.tensor_copy(invT[:], piT[:])
                nc.sync.dma_start(out=invFM[s][0:1, :], in_=invT[:])
                att = [tms.tile([128, C], BF16, tag=f"att{mh}", name=f"att{mh}")
                       for mh in range(2)]
                for mh in range(2):
                    rmax = tms.tile([128, 1], F32, tag="rmax")
                    nc.vector.tensor_reduce(rmax[:], pe[mh][:], axis=AX,
                                            op=OP.min)
                    tdiff = tms.tile([128, C], F32, tag="tdiff")
                    nc.vector.tensor_scalar(tdiff[:], pe[mh][:], rmax[:], 30.0,
                                            OP.subtract, OP.min)
                    ex = tms.tile([128, C], F32, tag="ex")
                    nc.scalar.activation(out=ex[:], in_=tdiff[:], func=ACTF.Exp,
                                         scale=-1.0)
                    rsum = tms.tile([128, 1], F32, tag="rsum")
                    nc.vector.reduce_sum(rsum[:], ex[:], axis=AX)
                    rinv = tms.tile([128, 1], F32, tag="rinv")
                    nc.vector.reciprocal(rinv[:], rsum[:])
                    nc.vector.tensor_scalar(att[mh][:], ex[:], rinv[:], gc,
                                            OP.mult, OP.mult)
                att8T = tms.tile([128, 2, C], FP8, tag="att8T", bufs=2)
                for mh in range(2):
                    for kt in range(2):
                        pat = psR.tile([128, 128], BF16, tag="rt")
                        nc.tensor.transpose(pat[:],
                                            att[mh][:, kt*128:(kt+1)*128],
                                            idb_sb[:])
                        nc.vector.tensor_copy(
                            att8T[:, kt, mh*128:(mh+1)*128], pat[:])
                st["att8T"] = att8T

            def light_post_piece(s, nt):
                pib = psA.tile([M, 512], F32, tag="mm")
                nc.tensor.matmul(pib[:], o32_sb[0:1, :],
                                 invFM[s][0:1, nt*512:(nt+1)*512],
                                 start=True, stop=True)
                nc.vector.tensor_tensor(Qap(s, nt*512, 512),
                                        Qap(s, nt*512, 512), pib[:],
                                        OP.mult)

            def light_chain_post(s, st):
                for nt in range(NT512):
                    light_post_piece(s, nt)

            def light_b(s, st, nts):
                """wv + catt in one PSUM; combine a = 2*z1 + psum.
                s=0: store into ax.  s=1: ay consumed on the fly ->
                u8[0] = fp8(x1*ay), u8[1] = fp8(y1*ax)."""
                z = z1[s]
                kv, att8T = st["kv"], st["att8T"]
                for nt in nts:
                    pos = []
                    for mh in range(2):
                        po = psA.tile([128, 512], F32, tag="mm")
                        nc.tensor.matmul(po[:], kv[:, mh*128:(mh+1)*128],
                                         Qap(s, nt*512, 512),
                                         start=True, stop=False)
                        nc.tensor.matmul(po[:],
                                         att8T[:, :, mh*128:(mh+1)*128],
                                         z8ap(s, nt*512, 512),
                                         start=False, stop=True,
                                         perf_mode=DROW)
                        pos.append(po)
                    for mh in range(2):
                        sl = slice(nt*512, (nt+1)*512)
                        if s == 0:
                            nc.vector.scalar_tensor_tensor(
                                ax[mh][:, sl], z[mh][:, sl], 2.0, pos[mh][:],
                                OP.mult, OP.add)
                            # u_y = y1*ax ready as soon as ax lands (Pool,
                            # idle in this phase) -> mlp-Y can run early
                            nc.gpsimd.tensor_tensor(u8[1][nt][:, mh, :],
                                                    z1[1][mh][:, sl],
                                                    ax[mh][:, sl], OP.mult)
                        else:
                            ay_t = tvcp.tile([128, 512], BF16, tag="ayt")
                            nc.vector.scalar_tensor_tensor(
                                ay_t[:], z[mh][:, sl], 2.0, pos[mh][:],
                                OP.mult, OP.add)
                            nc.gpsimd.tensor_tensor(u8[0][nt][:, mh, :],
                                                    z1[0][mh][:, sl], ay_t[:],
                                                    OP.mult)

            # mlp pieces -----------------------------------------------------
            def mlp_fc1(s, nt, min6_dve=3, relu_dve=0):
                """hidden = relu6(fc1(u8) + b): 2 DROW passes per 128-block
                (w1 hi + lo); relu+bias drains on Act (last `relu_dve`
                halves on DVE), min6 split DVE/Pool."""
                hps = []
                for p in range(4):
                    h8 = hp.tile([128, 2, 512], FP8, tag="h")
                    for i in range(2):
                        hm = 2*p + i
                        ph = psA.tile([128, 512], F32, tag="mm")
                        for wsb in (w1h_sb, w1l_sb):
                            nc.tensor.matmul(
                                ph[:], wsb[:, :, hm*128:(hm+1)*128],
                                u8[s][nt][:],
                                start=(wsb is w1h_sb), stop=(wsb is w1l_sb),
                                perf_mode=DROW)
                        if 2*p + i >= 8 - relu_dve:
                            nc.vector.tensor_scalar(
                                h8[:, i, :], ph[:], b1c_sb[:, hm:hm+1], 0.0,
                                OP.add, OP.max)
                        else:
                            nc.scalar.activation(out=h8[:, i, :], in_=ph[:],
                                                 func=ACTF.Relu,
                                                 bias=b1c_sb[:, hm:hm+1])
                    if p < min6_dve:
                        nc.vector.tensor_scalar(h8[:], h8[:], 6.0, None,
                                                OP.min)
                    else:
                        nc.gpsimd.tensor_scalar(h8[:], h8[:], 6.0, None,
                                                OP.min)
                    hps.append(h8)
                return hps

            def mlp_fc2(hps):
                outs = []
                for mh in range(2):
                    p2 = psL.tile([128, 512], F32, tag="lv")
                    for p in range(4):
                        for k, wsb in enumerate((w2h_sb, w2l_sb)):
                            nc.tensor.matmul(
                                p2[:], wsb[:, 2*p:2*p+2, mh*128:(mh+1)*128],
                                hps[p][:], start=(p == 0 and k == 0),
                                stop=(p == 3 and k == 1),
                                perf_mode=DROW)
                    outs.append(p2)
                return outs

            def mlp_y_unit(nt):
                """y2 = mlp(u_y); y3 = y + y2 -> DRAM (re-read in the x loop,
                same sync queue so the round trip is ordered)."""
                hs_y = mlp_fc1(1, nt, min6_dve=2, relu_dve=0)
                y2p = mlp_fc2(hs_y)
                for mh in range(2):
                    yin = tio.tile([128, 512], F32, tag="zin", bufs=6)
                    nc.sync.dma_start(out=yin[:],
                                      in_=d["yb"][mh*128:(mh+1)*128,
                                                  nt*512:(nt+1)*512])
                    y3t = tio.tile([128, 512], F32, tag="y3t", bufs=4)
                    nc.vector.scalar_tensor_tensor(
                        y3t[:], y2p[mh][:], b2c_sb[:, mh:mh+1], yin[:],
                        OP.add, OP.add)
                    nc.sync.dma_start(out=y3o[mh*128:(mh+1)*128,
                                              nt*512:(nt+1)*512], in_=y3t[:])

            st0, st1 = {}, {}
            sepconv_x()
            light_a(0, st0,
                    filler=lambda u: (pw_unit((u // 2) // 4,
                                              4 + (u // 2) % 4, alt_copy=True)
                                      if u % 2 == 0 else None),
                    prefill=lambda: [pw_unit(mh, nt, alt_copy=True)
                                     for mh in range(2) for nt in range(4)])
            light_chain(0, st0)
            sepconv_dw1()              # PE filler for chain(0) latency
            light_chain_post(0, st0)
            def la1_filler(u):
                if u % 4 == 0 and u < 12:
                    light_b(0, st0, [u // 4 + 3])
                elif u == 12:
                    mlp_y_unit(0)
            light_a(1, st1, filler=la1_filler,
                    prefill=lambda: light_b(0, st0, range(0, 3)))
            light_b(0, st0, range(6, NT512))
            light_chain(1, st1)
            for nt in range(1, NT512):  # mlp-Y fills chain(1) latency
                mlp_y_unit(nt)
                light_post_piece(1, nt - 1)
                if nt == 3:
                    light_b(1, st1, [0])
                elif nt == 5:
                    light_b(1, st1, [1])
            light_post_piece(1, NT512 - 1)

            # x-stream mlp + final adds, tile by tile
            for nt in range(NT512):
                if nt + 2 < NT512:
                    light_b(1, st1, [nt + 2])
                ins = []
                for mh in range(2):
                    y3in = tio.tile([128, 512], F32, tag="zin", bufs=6)
                    nc.sync.dma_start(out=y3in[:],
                                      in_=y3o[mh*128:(mh+1)*128,
                                              nt*512:(nt+1)*512])
                    xin = tio.tile([128, 512], F32, tag="zin", bufs=6)
                    nc.sync.dma_start(out=xin[:],
                                      in_=d["xb"][mh*128:(mh+1)*128,
                                                  nt*512:(nt+1)*512])
                    ins.append((y3in, xin))
                hs_x = mlp_fc1(0, nt, min6_dve=4)
                x2p = mlp_fc2(hs_x)
                for mh in range(2):
                    y3in, xin = ins[mh]
                    x3t = tio.tile([128, 512], F32, tag="x3t")
                    nc.vector.scalar_tensor_tensor(
                        x3t[:], x2p[mh][:], b2c_sb[:, mh:mh+1], y3in[:],
                        OP.add, OP.add)
                    nc.gpsimd.tensor_tensor(x3t[:], x3t[:], xin[:], OP.add)
                    nc.sync.dma_start(out=x3o[mh*128:(mh+1)*128,
                                              nt*512:(nt+1)*512], in_=x3t[:])
    nc.compile()
    return nc


def _hilo(a):
    hi = np.clip(a, -240.0, 240.0).astype(F8)
    lo = (a - hi.astype(np.float32)).astype(F8)
    return hi, lo


def _prep_weights(i):
    """Host-side weight folding; returns dict of DMA-ready arrays."""
    bf = lambda a: np.ascontiguousarray(a).astype(BF)
    f8 = lambda a: np.ascontiguousarray(a).astype(F8)
    f32 = lambda a: np.ascontiguousarray(a, dtype=np.float32)

    pw = f32(i["conv_pw_w"])            # [C, C]  out,in
    dw = f32(i["conv_dw_w"])[:, 0]      # [C, 3, 3]
    cs, ct = f32(i["conv_bn_s"]), f32(i["conv_bn_t"])
    ns, nt_ = f32(i["norm_s"]), f32(i["norm_t"])
    dwf = dw * (cs * ns)[:, None, None]                    # fold BN+norm scale
    zbias = ct * ns + nt_                                  # fold BN+norm shift
    dwd = np.zeros((128, 36, 128), np.float32)
    for half in range(2):
        for tap, (dy, dx) in enumerate(TAPS):
            w = dwf[half*128:(half+1)*128, dy+1, dx+1]
            whi, wlo = _hilo(w)
            base = (half*9 + tap) * 2
            dwd[np.arange(128), base, np.arange(128)] = whi.astype(np.float32)
            dwd[np.arange(128), base+1, np.arange(128)] = wlo.astype(np.float32)
    tv = np.concatenate([np.eye(C, dtype=np.float32), f32(i["v_w"]).T], axis=1)
    tv8 = tv.reshape(2, 128, 512).transpose(1, 0, 2)       # [128, kt, 512]
    s1, t1 = f32(i["bn1_s"]), f32(i["bn1_t"])
    s2, t2 = f32(i["bn2_s"]), f32(i["bn2_t"])
    fc1, fb1 = f32(i["fc1_w"]), f32(i["fc1_b"])
    fc2, fb2 = f32(i["fc2_w"]), f32(i["fc2_b"])
    w1f = (s1[:, None] * fc1) * ns[None, :]
    b1f = s1 * (fc1 @ nt_ + fb1) + t1
    w2f = s2[:, None] * fc2
    b2f = s2 * fb2 + t2

    w1T = w1f.T.reshape(2, 128, HID).transpose(1, 0, 2)    # [128, kt, HID]
    w1h, w1l = _hilo(w1T)
    w2T = w2f.T.reshape(8, 128, C).transpose(1, 0, 2)      # [128, k8, C]
    w2h, w2l = _hilo(w2T)
    qT8 = f32(i["q_w"]).T.reshape(2, 128, M).transpose(1, 0, 2)

    return {
        "pwT": bf(pw.T),
        "dwd8": f8(dwd),
        "zbc": f32(zbias.reshape(2, 128).T),
        "tv8": f8(tv8),
        "kT": bf(f32(i["k_w"]).T),
        "qT8": f8(qT8),
        "kT8w": f8(f32(i["k_w"]).T.reshape(2, 128, M).transpose(1, 0, 2)),
        "w1h8": w1h, "w1l8": w1l,
        "b1c": f32(b1f.reshape(8, 128).T),
        "w2h8": w2h, "w2l8": w2l,
        "b2c": f32(b2f.reshape(2, 128).T),
        "i128f": np.eye(128, dtype=np.float32),
        "i128b": bf(np.eye(128)),
        "o128f8": f8(np.ones((128, 1))),
        "ones32": bf(np.ones((1, M))),
    }


def kernel(**inputs):
    x = np.ascontiguousarray(inputs["x"], dtype=np.float32)
    y = np.ascontiguousarray(inputs["y"], dtype=np.float32)
    B = x.shape[0]
    gk = float(np.asarray(inputs["gamma_k"]).reshape(-1)[0])
    gc = float(np.asarray(inputs["gamma_c"]).reshape(-1)[0])

    wmaps = _prep_weights(inputs)
    key = (gk, gc) + tuple(hash(v.tobytes()) for _, v in sorted(wmaps.items()))
    if key not in _CACHE:
        _CACHE.clear()
        _CACHE[key] = _build(gk, gc)
    nc = _CACHE[key]

    in_maps = []
    for b in range(B):
        m = dict(wmaps)
        m["xb"] = x[b].reshape(C, N)
        m["yb"] = y[b].reshape(C, N)
        m["xb16"] = x[b].reshape(C, N).astype(BF)
        m["yb16"] = y[b].reshape(C, N).astype(BF)
        in_maps.append(m)

    res = run_bass_kernel_spmd(nc, in_maps, list(range(B)))
    x3 = np.stack([res.results[b]["x3o"].reshape(C, H, W) for b in range(B)])
    y3 = np.stack([res.results[b]["y3o"].reshape(C, H, W) for b in range(B)])
    return (x3.astype(np.float32), y3.astype(np.float32))
